# revision 55
# baseline (speedup 1.0000x reference)
"""Trainium2 Bass kernel for nn_Attention (RMSNorm + QKV + 16-head attention + out-proj).

Sharding: 8 cores = 4 batches x 2 head-groups (DP x TP). Each core gets one
batch element and 8 of the 16 heads, computes a partial out-projection
([2048, 1024]); the host sums the two head-group partials per batch.

Per-core pipeline (T=2048 tokens, D=1024; all matmul operands bf16 with fp32
PSUM accumulation; measured end-to-end error is ~6e-3 scale-relative):
  Front (fused per token tile, so PE streams matmuls back-to-back while
      ScalarE does the RMS stats and DVE the casts/copies): load x,
      RMS-normalize (gamma*sqrt(D)*dh^-0.5*log2e folded into the weights on
      the host), 8 PE-transposes into one PSUM bank then a single strided
      copy to feature-major xnT [128, 8fc, T]; then the v projection for
      that tile (ones column appended per head so the AV matmul, M=65, also
      produces the softmax denominator in row 64 for free); after each
      512-token chunk, the q/k projections for that chunk.
  P3  attention per (pair, 512-wide q chunk): S^T tiles [128 keys, 2x512]
      with the two heads row-packed on the PE (K=64 at array rows 0-63 /
      64-127); one ScalarE exp over both banks (exp(ln2*x)=2^x via the free
      affine since log2e is folded into wq); AV accumulates per head into
      separate banks; 1/denom is broadcast across partitions with a tiny
      f32r ones-matmul; normalization tails are emitted one group late so
      their PE work never head-of-line-blocks the S stream; head-1 results
      are DMA-shifted to partitions 64:127 for the out-projection pair tile.
  P4  out-projection: matmul(lhsT=attn pair tile, rhs=w_out rows),
      accumulated over the 4 pairs in PSUM.

Measured on HW (loop-delta): phase fusion took the kernel 619us -> 443us on
a quiet device. A DVE exp offload (Schraudolph 2^x), deeper PSUM buffering
variants, SBUF-side tails, and 2-group slot interleaving were all tried and
benchmarked slower or neutral on HW; knobs remain in _CFG.

Toolchain workarounds: sync waits are capped at 1 per instruction (excess
moved onto NoOps via a BIR JSON post-pass) because this walrus rejects
multi-wait encodings; fp32r is used only where precision matters (1/denom
broadcast); gpsimd custom ops and DMA partition-broadcast are unavailable.
"""

import json
import numpy as np

B, T, D = 4, 2048, 1024
HEADS, DH = 16, 64
NT = T // 128   # 16 token tiles
FC = D // 128   # 8 feature chunks
NPAIR = 4       # head pairs per core (8 heads)
QCN = 4         # q chunks of 512
KT = NT         # key tiles

_PROG = {}

# Tuning knobs (read at program-build time; _PROG cache key includes them).
_CFG = {
    "dve_kt": (),  # key tiles whose exp runs on DVE (offload hurt on HW)
    "pi_copy": 0,  # 1: copy bitcast p through a bf16 tile before the AV MM
    "av_lag": 2,   # slots by which DVE-slot AV matmuls are issued late
    "s_bufs": 2,   # ps_s PSUM double/triple buffering (2 banks each)
    "av_bufs": 2,  # ps_av buffering (2 banks each)
    "exp_fd": 1024,  # 512 = timing probe with half the ScalarE exp work
    "s_merge": 0,  # 1: zero-padded qT + single N=1024 S matmul (ISA-illegal)
    "p4_wide": 0,  # 1: P4 matmuls at N=1024 (ISA-illegal: out spans 2 banks)
    "no_s2": 0,      # probe: skip the second S matmul (head1 scores garbage)
    "av_single": 0,  # probe: single AV matmul per slot (head1 out garbage)
    "cheap_tail": 0,  # probe: minimal tail (no normalization mults)
    "pp_bufs": 8,    # p_pool depth
    "deep": 0,       # 1: deepen SBUF-side pools (bsb/stg/rcp/xp/acc/pst)
    "streams": 1,    # 2: interleave two attention groups slot-by-slot
    "p4_pair": 0,    # 1: P4 hf-inner loop so lhsT is reused (LDW dedup)
}


def _cfg_key():
    return tuple(sorted((k, tuple(v) if isinstance(v, (list, tuple)) else v)
                        for k, v in _CFG.items()))

# ---------------------------------------------------------------------------
# BIR post-pass: this walrus build rejects >1 sync wait per instruction in
# some encodings; move excess waits onto NoOps inserted before the offender.
_MAX_WAITS = 2
# opcodes whose walrus encoding only fits one sync wait
_ONE_WAIT_OPS = ()


def _split_excess_waits(bir_json: bytes) -> bytes:
    d = json.loads(bir_json)
    changed = False
    for fn in d.get("functions", []):
        for blk in fn.get("blocks", []):
            new_insts = []
            for inst in blk.get("instructions", []):
                si = inst.get("sync_info") or {}
                waits = si.get("on_wait") or []
                _MAX_WAITS = 1
                if len(waits) > _MAX_WAITS:
                    changed = True
                    excess = waits[: len(waits) - _MAX_WAITS]
                    si["on_wait"] = waits[len(waits) - _MAX_WAITS:]
                    inst["sync_info"] = si
                    for k in range(0, len(excess), _MAX_WAITS):
                        new_insts.append({
                            "debug": inst.get("debug", 0),
                            "engine": inst["engine"],
                            "ins": [],
                            "name": f"{inst['name']}-wsplit{k}",
                            "opcode": "NoOp",
                            "outs": [],
                            "sync_info": {
                                "on_update": [],
                                "on_wait": excess[k : k + _MAX_WAITS],
                            },
                        })
                new_insts.append(inst)
            blk["instructions"] = new_insts
    if not changed:
        return bir_json
    return json.dumps(d).encode()


def _install_bir_patch():
    import concourse.bass2jax as bass2jax
    import concourse.bass_utils as bass_utils

    if getattr(bass2jax.compile_bir_kernel, "_is_waitsplit_patch", False):
        return
    orig = bass_utils.compile_bir_kernel

    def patched(bir_json, tmpdir, neff_name="file.neff"):
        return orig(_split_excess_waits(bir_json), tmpdir, neff_name)

    patched._is_waitsplit_patch = True
    bass2jax.compile_bir_kernel = patched
    bass_utils.compile_bir_kernel = patched


# ---------------------------------------------------------------------------


def _build_program(loop_n=1, phases="all"):
    from contextlib import ExitStack

    import concourse.bass as bass
    import concourse.mybir as mybir
    import concourse.tile as tile
    from concourse.masks import make_identity

    F32 = mybir.dt.float32
    F32R = mybir.dt.float32r
    BF16 = mybir.dt.bfloat16
    I16 = mybir.dt.int16
    AF = mybir.ActivationFunctionType
    OP = mybir.AluOpType

    # exp work split across engines: key tiles in DVE_KT use the DVE
    # Schraudolph 2^x; the rest use ScalarE exp (scale=ln2).
    DVE_KT = frozenset(_CFG["dve_kt"])
    _SCHRAUDOLPH_BIAS = float(127 * 128 - 5.6)
    _LN2 = float(np.log(2.0))

    nc = bass.Bass("TRN2", target_bir_lowering=False, debug=False, num_devices=8)
    x_d = nc.dram_tensor("x", [T, D], F32, kind="ExternalInput").ap()
    wq_d = nc.dram_tensor("wq", [128, FC, 512], BF16, kind="ExternalInput").ap()
    wk_d = nc.dram_tensor("wk", [128, FC, 512], BF16, kind="ExternalInput").ap()
    wv_d = nc.dram_tensor("wv", [128, FC, 512], BF16, kind="ExternalInput").ap()
    wo_d = nc.dram_tensor("wo", [128, NPAIR, D], BF16, kind="ExternalInput").ap()
    out_d = nc.dram_tensor("out", [T, D], F32, kind="ExternalOutput").ap()

    with tile.TileContext(nc) as tc:
        with ExitStack() as es:
            singles = es.enter_context(tc.tile_pool(name="singles", bufs=1))
            qpool = es.enter_context(tc.tile_pool(name="qp", bufs=1))
            kpool = es.enter_context(tc.tile_pool(name="kp", bufs=1))
            vpool = es.enter_context(tc.tile_pool(name="vp", bufs=1))

            ident = singles.tile([128, 128], F32)
            make_identity(nc, ident[:])
            ident_bf = singles.tile([128, 128], BF16)
            nc.vector.tensor_copy(ident_bf[:], ident[:])
            ones_f32 = singles.tile([128, 64], F32)
            nc.gpsimd.memset(ones_f32[:], 1.0)
            ones_r = singles.tile([128, 64], F32R)
            nc.vector.tensor_copy(ones_r[:], ones_f32[:])
            stats = singles.tile([128, 64], F32)
            sqscratch = singles.tile([128, D], F32)

            # zero-padded qT for the merged S matmul: head h's dims live in
            # rows h*64:(h+1)*64 of slot h with the other half zero, so one
            # K=128 N=1024 matmul computes both heads without mixing them.
            if _CFG["s_merge"]:
                qTp = qpool.tile([128, NPAIR, QCN, 2, 512], BF16)
                nc.vector.memset(qTp[:], 0.0)

            # v with a ones column appended per head: AV matmul with M=65
            # yields attn_out rows 0:64 plus the softmax denominator in row 64
            v_sb = vpool.tile([128, NT, NPAIR, 2, 65], BF16)
            nc.vector.tensor_copy(
                v_sb[:, :, :, :, 64:65],
                ones_f32[:, 0:1].broadcast_to([128, NT, NPAIR, 2, 1]))
            aopool = es.enter_context(tc.tile_pool(name="aout", bufs=1))
            attn_sb = aopool.tile([128, NPAIR, T], BF16)

            import contextlib
            loop_ctx = (tc.For_i(0, loop_n, 1) if loop_n > 1
                        else contextlib.nullcontext())
            with loop_ctx:
                # SBUF frame that is released before the attention phase
                xnt_es = es.enter_context(ExitStack())
                xnt_pool = xnt_es.enter_context(tc.tile_pool(name="xnt", bufs=1))
                xnT = xnt_pool.tile([128, FC, T], BF16)

                if not _CFG["s_merge"]:
                    qTf = qpool.tile([128, NPAIR, T], BF16)
                kTf = kpool.tile([128, NPAIR, T], BF16)

                # ---- Fused front: per token tile, RMS-normalize + transpose
                # (P1) then the v projection (P2a); after each 512-token chunk
                # completes, the q/k projections for that chunk (P2b). Keeps
                # PE streaming back-to-back while ACT does the RMS stats and
                # DVE the casts/copies.
                front_es = es.enter_context(ExitStack())
                ps_t = front_es.enter_context(
                    tc.tile_pool(name="ps_t", bufs=3 if _CFG["deep"] else 2,
                                 space="PSUM"))
                ps_acc = front_es.enter_context(
                    tc.tile_pool(name="ps_acc", bufs=4 if _CFG["deep"] else 3,
                                 space="PSUM"))
                wqkv = front_es.enter_context(tc.tile_pool(name="wqkv", bufs=1))
                xp = front_es.enter_context(
                    tc.tile_pool(name="xin", bufs=4 if _CFG["deep"] else 3))
                wv_sb = wqkv.tile([128, FC, 512], BF16)
                nc.sync.dma_start(wv_sb[:], wv_d[:])
                wq_sb = wqkv.tile([128, FC, 512], BF16)
                nc.sync.dma_start(wq_sb[:], wq_d[:])
                wk_sb = wqkv.tile([128, FC, 512], BF16)
                nc.sync.dma_start(wk_sb[:], wk_d[:])
                for tt in range(NT):
                    x_t = xp.tile([128, D], F32, tag="x")
                    nc.sync.dma_start(x_t[:], x_d[tt * 128:(tt + 1) * 128, :])
                    ss = stats[:, tt:tt + 1]
                    nc.scalar.activation(
                        sqscratch[:], x_t[:], AF.Square, accum_out=ss)
                    nrm = stats[:, 16 + tt:17 + tt]
                    nc.scalar.sqrt(nrm, ss)
                    nc.vector.tensor_scalar_max(nrm, nrm, 1e-12)
                    rinv = stats[:, 32 + tt:33 + tt]
                    nc.vector.reciprocal(rinv, nrm)
                    xn_b = xp.tile([128, D], BF16, tag="xb")
                    nc.vector.tensor_scalar_mul(xn_b[:], x_t[:], rinv)
                    pt = ps_t.tile([128, D], BF16, tag="t")
                    for fc in range(FC):
                        nc.tensor.transpose(
                            pt[:, fc * 128:(fc + 1) * 128],
                            xn_b[:, fc * 128:(fc + 1) * 128], ident_bf[:])
                    nc.vector.tensor_copy(
                        xnT[:, :, tt * 128:(tt + 1) * 128],
                        pt[:].rearrange("p (f c) -> p f c", f=FC))
                    # P2a: v for this token tile
                    pv = ps_acc.tile([128, 512], F32, tag="acc")
                    for fc in range(FC):
                        nc.tensor.matmul(
                            pv[:], xnT[:, fc, tt * 128:(tt + 1) * 128],
                            wv_sb[:, fc, :],
                            start=(fc == 0), stop=(fc == FC - 1))
                    nc.vector.tensor_copy(
                        v_sb[:, tt, :, :, 0:64],
                        pv[:].rearrange("p (pr h c) -> p pr h c", pr=NPAIR, h=2))
                    # P2b: q/k for the completed 512-token chunk
                    if tt % 4 == 3:
                        qc = tt // 4
                        cs = slice(qc * 512, (qc + 1) * 512)
                        for p in range(NPAIR):
                            pq = ps_acc.tile([128, 512], F32, tag="acc")
                            for fc in range(FC):
                                nc.tensor.matmul(
                                    pq[:], wq_sb[:, fc, p * 128:(p + 1) * 128],
                                    xnT[:, fc, cs],
                                    start=(fc == 0), stop=(fc == FC - 1))
                            if _CFG["s_merge"]:
                                nc.vector.tensor_copy(
                                    qTp[0:64, p, qc, 0, :], pq[0:64, :])
                                nc.scalar.copy(
                                    qTp[64:128, p, qc, 1, :], pq[64:128, :])
                            else:
                                nc.vector.tensor_copy(qTf[:, p, cs], pq[:])
                            pk = ps_acc.tile([128, 512], F32, tag="acc")
                            for fc in range(FC):
                                nc.tensor.matmul(
                                    pk[:], wk_sb[:, fc, p * 128:(p + 1) * 128],
                                    xnT[:, fc, cs],
                                    start=(fc == 0), stop=(fc == FC - 1))
                            nc.scalar.copy(kTf[:, p, cs], pk[:])
                front_es.close()
                xnt_es.close()

                # ---- P3: attention; AV double-buffered, normalization tails
                # lagged one group so their PE work never blocks the S stream
                att_es = es.enter_context(ExitStack())
                ps_s = att_es.enter_context(
                    tc.tile_pool(name="ps_s", bufs=_CFG["s_bufs"], space="PSUM"))
                ps_av = att_es.enter_context(
                    tc.tile_pool(name="ps_av", bufs=_CFG["av_bufs"],
                                 space="PSUM"))
                p_pool = att_es.enter_context(
                    tc.tile_pool(name="pp", bufs=_CFG["pp_bufs"]))
                pi_pool = att_es.enter_context(tc.tile_pool(name="pip", bufs=3))
                av_pool = att_es.enter_context(tc.tile_pool(name="avs", bufs=2))
                _dp = 4 if _CFG["deep"] else 2
                rcp_pool = att_es.enter_context(
                    tc.tile_pool(name="rcp", bufs=_dp))
                bsb_pool = att_es.enter_context(
                    tc.tile_pool(name="bsb", bufs=_dp))
                stg_pool = att_es.enter_context(
                    tc.tile_pool(name="stg", bufs=_dp))

                def emit_rcp(pAV):
                    if _CFG["cheap_tail"]:
                        return None
                    # reciprocal of the denominator row; issued as soon as the
                    # AV group stops so the lagged pB matmuls never wait on DVE
                    rcp = rcp_pool.tile([65, 1024], F32R, tag="rcp")
                    with nc.allow_low_precision(reason="1/denom feeds f32r matmul"):
                        nc.vector.reciprocal(rcp[64:65, :], pAV[64:65, :])
                    return rcp

                def emit_tail(p, qc, pAV, rcp):
                    cs = slice(qc * 512, (qc + 1) * 512)
                    if _CFG["cheap_tail"]:
                        # probe: unnormalized single-copy tail
                        nc.scalar.copy(attn_sb[0:64, p, cs], pAV[0:64, 0:512])
                        return
                    pBa = ps_s.tile([128, 1024], F32, tag="s")
                    nc.tensor.matmul(
                        pBa[0:64, 0:512], ones_r[64:65, :], rcp[64:65, 0:512],
                        start=True, stop=True, tile_position=(64, 0))
                    nc.tensor.matmul(
                        pBa[0:64, 512:1024], ones_r[64:65, :], rcp[64:65, 512:1024],
                        start=True, stop=True, tile_position=(64, 0))
                    bsb = bsb_pool.tile([64, 1024], F32, tag="b")
                    nc.vector.tensor_copy(bsb[:], pBa[0:64, :])
                    nc.vector.tensor_tensor(
                        attn_sb[0:64, p, cs], pAV[0:64, 0:512], bsb[:, 0:512],
                        OP.mult)
                    stg = stg_pool.tile([64, 512], BF16, tag="stg")
                    nc.vector.tensor_tensor(
                        stg[:], pAV[0:64, 512:1024], bsb[:, 512:1024], OP.mult)
                    nc.sync.dma_start(attn_sb[64:128, p, cs], stg[:])

                # Precompute the AV issue schedule: DVE-slot AVs are issued
                # av_lag slots late so the in-order PE stream never
                # head-of-line-blocks on the DVE exp. (PSUM accumulation is
                # order-independent; start/stop go on the first/last ISSUED.)
                AV_LAG = _CFG["av_lag"]
                issue_seq = []   # kt values in AV issue order
                lag_sched = []   # per slot: list of lagged kt issued there
                _q = []
                for kt in range(KT):
                    due = [l for l in _q if l[1] <= kt]
                    _q = [l for l in _q if l[1] > kt]
                    here = [l[0] for l in due]
                    if kt in DVE_KT:
                        _q.append((kt, kt + AV_LAG))
                    else:
                        here.append(kt)
                    lag_sched.append(here)
                    issue_seq.extend(here)
                tail_flush = [l[0] for l in _q]
                issue_seq.extend(tail_flush)
                first_av, last_av = issue_seq[0], issue_seq[-1]

                if _CFG["streams"] == 2 and phases != "front":
                    # Two groups interleaved slot-by-slot: each group's
                    # S->exp->AV chain gets a full slot-pair of slack while
                    # the other group keeps the engines fed. Same 8 PSUM
                    # banks (each group single-buffers S and AV).
                    groups = [(p, qc) for p in range(NPAIR)
                              for qc in range(QCN)]
                    pend = []

                    def s2_av(c, kt, start, stop):
                        p, qc = c["p"], c["qc"]
                        p_t = c.pop("pt_" + str(kt))
                        nc.tensor.matmul(
                            c["pAV"][0:65, 0:512],
                            v_sb[:, kt, p, 0, :], p_t[:, 0:512],
                            start=start, stop=stop)
                        nc.tensor.matmul(
                            c["pAV"][0:65, 512:1024],
                            v_sb[:, kt, p, 1, :], p_t[:, 512:1024],
                            start=start, stop=stop)

                    for blk in range(0, len(groups), 2):
                        ctxs = []
                        for (p, qc) in groups[blk:blk + 2]:
                            pAV = ps_av.tile(
                                [128, 1024], F32, tag="av", name="pAV")
                            ctxs.append({"p": p, "qc": qc, "pAV": pAV})
                        for kt in range(KT):
                            for c in ctxs:
                                p, qc = c["p"], c["qc"]
                                cs = slice(qc * 512, (qc + 1) * 512)
                                ks = slice(kt * 128, (kt + 1) * 128)
                                kT = kTf[:, p, :]
                                qT = qTf[:, p, :]
                                pS = ps_s.tile([128, 1024], F32, tag="s")
                                nc.tensor.matmul(
                                    pS[:, 0:512], kT[0:64, ks], qT[0:64, cs],
                                    start=True, stop=True,
                                    tile_position=(0, 0))
                                nc.tensor.matmul(
                                    pS[:, 512:1024], kT[64:128, ks],
                                    qT[64:128, cs],
                                    start=True, stop=True,
                                    tile_position=(64, 0))
                                c["pS"] = pS
                            if kt == 0 and pend:
                                emit_tail(*pend.pop(0))
                            if kt == 1 and pend:
                                emit_tail(*pend.pop(0))
                            for c in ctxs:
                                if kt > 0:
                                    s2_av(c, kt - 1, kt - 1 == 0, False)
                            for c in ctxs:
                                p_tile = p_pool.tile(
                                    [128, 1024], BF16, tag="P")
                                nc.scalar.activation(
                                    p_tile[:], c["pS"], AF.Exp, scale=_LN2)
                                c["pt_" + str(kt)] = p_tile[:]
                        for c in ctxs:
                            s2_av(c, KT - 1, False, True)
                        for c in ctxs:
                            rcp = emit_rcp(c["pAV"])
                            pend.append((c["p"], c["qc"], c["pAV"], rcp))
                    for t in pend:
                        emit_tail(*t)

                pending = None
                for p in range(NPAIR if (phases != "front"
                                         and _CFG["streams"] == 1) else 0):
                    qT = None if _CFG["s_merge"] else qTf[:, p, :]
                    kT = kTf[:, p, :]
                    for qc in range(QCN):
                        cs = slice(qc * 512, (qc + 1) * 512)
                        pAV = ps_av.tile([128, 1024], F32, tag="av")
                        p_aps = {}

                        def emit_av(kt):
                            st = kt == first_av
                            sp = kt == last_av
                            p0, p1 = p_aps.pop(kt)
                            nc.tensor.matmul(
                                pAV[0:65, 0:512],
                                v_sb[:, kt, p, 0, :], p0,
                                start=st, stop=sp)
                            if _CFG["av_single"]:
                                return
                            nc.tensor.matmul(
                                pAV[0:65, 512:1024],
                                v_sb[:, kt, p, 1, :], p1,
                                start=st, stop=sp)

                        for kt in range(KT):
                            ks = slice(kt * 128, (kt + 1) * 128)
                            pS = ps_s.tile([128, 1024], F32, tag="s")
                            if _CFG["no_s2"]:
                                nc.tensor.matmul(
                                    pS[:, 0:512], kT[0:64, ks], qT[0:64, cs],
                                    start=True, stop=True,
                                    tile_position=(0, 0))
                            elif _CFG["s_merge"]:
                                # one K=128 N=1024 matmul; the zero padding in
                                # qTp keeps the heads separate
                                nc.tensor.matmul(
                                    pS[:], kT[:, ks],
                                    qTp[:, p, qc, :, :].rearrange(
                                        "p a b -> p (a b)"),
                                    start=True, stop=True)
                            else:
                                nc.tensor.matmul(
                                    pS[:, 0:512], kT[0:64, ks], qT[0:64, cs],
                                    start=True, stop=True, tile_position=(0, 0))
                                nc.tensor.matmul(
                                    pS[:, 512:1024], kT[64:128, ks],
                                    qT[64:128, cs],
                                    start=True, stop=True,
                                    tile_position=(64, 0))
                            # S is in log2 domain (log2e folded into wq).
                            # ScalarE slots: exp(ln2*x) = 2^x via the free
                            # affine. DVE slots: Schraudolph 2^x -- int16
                            # round(128x + 16256 - C) bitcast to bf16 (max
                            # ~3.3% exp err on ~1/3 of key blocks; softmax
                            # denominators stay consistent via the ones
                            # column in v).
                            if kt in DVE_KT:
                                pi = pi_pool.tile([128, 1024], I16, tag="Pi")
                                nc.vector.tensor_scalar(
                                    pi[:], pS[:], 128.0, _SCHRAUDOLPH_BIAS,
                                    OP.mult, OP.add)
                                if _CFG["pi_copy"]:
                                    p_tile = p_pool.tile(
                                        [128, 1024], BF16, tag="P")
                                    nc.vector.tensor_copy(
                                        p_tile[:], pi[:].bitcast(BF16))
                                    pb = p_tile[:]
                                else:
                                    pb = pi[:].bitcast(BF16)
                                p_aps[kt] = (pb[:, 0:512], pb[:, 512:1024])
                            elif _CFG["exp_fd"] == 512:
                                # timing probe: half the ScalarE exp work
                                # (math wrong; head1 reuses head0's p)
                                p_tile = p_pool.tile([128, 1024], BF16, tag="P")
                                nc.scalar.activation(
                                    p_tile[:, 0:512], pS[:, 0:512],
                                    AF.Exp, scale=_LN2)
                                p_aps[kt] = (p_tile[:, 0:512],
                                             p_tile[:, 0:512])
                            else:
                                p_tile = p_pool.tile([128, 1024], BF16, tag="P")
                                nc.scalar.activation(
                                    p_tile[:], pS[:], AF.Exp, scale=_LN2)
                                p_aps[kt] = (p_tile[:, 0:512],
                                             p_tile[:, 512:1024])
                            # AV with ones-augmented v: rows 0:64 attn out,
                            # row 64 the softmax denominator (bank per head)
                            for lkt in lag_sched[kt]:
                                emit_av(lkt)
                            if kt == 4 and pending is not None:
                                emit_tail(*pending)
                                pending = None
                        for lkt in tail_flush:
                            emit_av(lkt)
                        rcp = emit_rcp(pAV)
                        pending = (p, qc, pAV, rcp)
                if pending is not None:
                    emit_tail(*pending)
                att_es.close()

                # ---- P4: out projection (accumulate over the 4 pairs)
                with tc.tile_pool(name="ps_o", bufs=3, space="PSUM") as ps_o, \
                     tc.tile_pool(name="wop", bufs=1) as wop, \
                     tc.tile_pool(name="osb", bufs=3) as osb:
                    wo_sb = wop.tile([128, NPAIR, D], BF16)
                    nc.sync.dma_start(wo_sb[:], wo_d[:])
                    for tt in range(NT):
                        if phases == "front":
                            # consume q/k/v so the front isn't dead code
                            qsrc = (qTp[:, tt % NPAIR, 0, 0, :]
                                    if _CFG["s_merge"] else
                                    qTf[:, tt % NPAIR, 0:512])
                            dm = osb.tile([128, 512], F32, tag="o")
                            nc.vector.tensor_tensor(
                                dm[:], qsrc,
                                kTf[:, tt % NPAIR, 0:512], OP.mult)
                            nc.vector.tensor_tensor(
                                dm[:, 0:65], v_sb[:, tt, 0, 0, :], dm[:, 0:65],
                                OP.mult)
                            nc.sync.dma_start(
                                out_d[tt * 128:(tt + 1) * 128, 0:512], dm[:])
                            continue
                        if phases != "all":
                            # consume attn_sb so attention isn't dead code
                            dm = osb.tile([128, 512], F32, tag="o")
                            nc.vector.tensor_copy(
                                dm[:], attn_sb[:, tt % NPAIR, 0:512])
                            nc.sync.dma_start(
                                out_d[tt * 128:(tt + 1) * 128, 0:512], dm[:])
                            continue
                        if _CFG["p4_wide"]:
                            po = ps_o.tile([128, 1024], F32, tag="ow")
                            for p in range(NPAIR):
                                nc.tensor.matmul(
                                    po[:], attn_sb[:, p, tt * 128:(tt + 1) * 128],
                                    wo_sb[:, p, :],
                                    start=(p == 0), stop=(p == NPAIR - 1))
                            o_sb = osb.tile([128, 1024], F32, tag="ow")
                            if tt % 2 == 0:
                                nc.vector.tensor_copy(o_sb[:], po[:])
                            else:
                                nc.scalar.copy(o_sb[:], po[:])
                            nc.sync.dma_start(
                                out_d[tt * 128:(tt + 1) * 128, :], o_sb[:])
                            continue
                        for hf in range(2):
                            po = ps_o.tile([128, 512], F32, tag="o")
                            for p in range(NPAIR):
                                nc.tensor.matmul(
                                    po[:], attn_sb[:, p, tt * 128:(tt + 1) * 128],
                                    wo_sb[:, p, hf * 512:(hf + 1) * 512],
                                    start=(p == 0), stop=(p == NPAIR - 1))
                            o_sb = osb.tile([128, 512], F32, tag="o")
                            nc.vector.tensor_copy(o_sb[:], po[:])
                            nc.sync.dma_start(
                                out_d[tt * 128:(tt + 1) * 128, hf * 512:(hf + 1) * 512],
                                o_sb[:])
    return nc


def _get_program(loop_n=1, phases="all"):
    key = (loop_n, phases, _cfg_key())
    if key not in _PROG:
        _install_bir_patch()
        _PROG[key] = _build_program(loop_n, phases)
    return _PROG[key]


def _make_in_maps(x, gamma, w_qkv, w_out):
    x = np.asarray(x, dtype=np.float32)
    gamma = np.asarray(gamma, dtype=np.float32)
    w_qkv = np.asarray(w_qkv, dtype=np.float32)
    w_out = np.asarray(w_out, dtype=np.float32)

    scale = gamma * np.float32(np.sqrt(D))          # fold sqrt(D)*gamma
    in_maps = []
    for core in range(8):
        b = core // 2
        hg = core % 2
        cols = slice(hg * 512, (hg + 1) * 512)
        wq = (w_qkv[:, 0 * D:1 * D][:, cols] * scale[:, None]
              * np.float32(DH ** -0.5) * np.float32(np.log2(np.e)))
        wk = w_qkv[:, 1 * D:2 * D][:, cols] * scale[:, None]
        wv = w_qkv[:, 2 * D:3 * D][:, cols] * scale[:, None]
        wo = w_out[hg * 512:(hg + 1) * 512, :]
        import ml_dtypes
        bf16 = ml_dtypes.bfloat16
        in_maps.append({
            "x": np.ascontiguousarray(x[b]),
            "wq": np.ascontiguousarray(
                wq.reshape(FC, 128, 512).transpose(1, 0, 2).astype(bf16)),
            "wk": np.ascontiguousarray(
                wk.reshape(FC, 128, 512).transpose(1, 0, 2).astype(bf16)),
            "wv": np.ascontiguousarray(
                wv.reshape(FC, 128, 512).transpose(1, 0, 2).astype(bf16)),
            "wo": np.ascontiguousarray(
                wo.reshape(NPAIR, 128, D).transpose(1, 0, 2).astype(bf16)),
        })
    return in_maps


_RUNNER = None


def _build_runner(nc):
    """Persistent jitted callable over the 8-core mesh (avoids per-call
    re-tracing that run_bass_kernel_spmd incurs)."""
    import jax
    import concourse.mybir as mybir
    from jax.sharding import Mesh, PartitionSpec
    from jax.experimental.shard_map import shard_map
    from concourse.bass2jax import (
        _bass_exec_p, install_neuronx_cc_hook, partition_id_tensor)

    install_neuronx_cc_hook()
    partition_name = nc.partition_id_tensor.name if nc.partition_id_tensor else None
    in_names, out_names, out_avals, zero_shapes = [], [], [], []
    for alloc in nc.m.functions[0].allocations:
        if not isinstance(alloc, mybir.MemoryLocationSet):
            continue
        name = alloc.memorylocations[0].name
        if alloc.kind == "ExternalInput":
            if name != partition_name:
                in_names.append(name)
        elif alloc.kind == "ExternalOutput":
            out_names.append(name)
            shape = tuple(alloc.tensor_shape)
            dtype = mybir.dt.np(alloc.dtype)
            out_avals.append(jax.core.ShapedArray(shape, dtype))
            zero_shapes.append((shape, dtype))
    n_params = len(in_names)
    all_in_names = tuple(in_names + out_names)
    if partition_name is not None:
        all_in_names = all_in_names + (partition_name,)

    def _body(*args):
        operands = list(args)
        if partition_name is not None:
            operands.append(partition_id_tensor())
        return tuple(_bass_exec_p.bind(
            *operands,
            out_avals=tuple(out_avals),
            in_names=all_in_names,
            out_names=tuple(out_names),
            lowering_input_output_aliases=(),
            sim_require_finite=True,
            sim_require_nnan=True,
            nc=nc,
        ))

    devices = jax.devices()[:8]
    mesh = Mesh(np.asarray(devices), ("core",))
    nin = n_params + len(out_names)
    fn = jax.jit(shard_map(
        _body, mesh=mesh, in_specs=(PartitionSpec("core"),) * nin,
        out_specs=(PartitionSpec("core"),) * len(out_names), check_rep=False))

    def runner(in_maps):
        args = [np.concatenate([np.asarray(in_maps[c][nm]) for c in range(8)],
                               axis=0) for nm in in_names]
        for shape, dtype in zero_shapes:
            args.append(np.zeros((8 * shape[0], *shape[1:]), dtype))
        outs = fn(*args)
        o = np.asarray(outs[0]).reshape(8, T, D)
        return [o[c] for c in range(8)]

    return runner


def run(x, gamma, w_qkv, w_out, trace=False):
    """Run on the 8 NeuronCores; returns (output, results-or-None)."""
    global _RUNNER
    nc = _get_program()
    in_maps = _make_in_maps(x, gamma, w_qkv, w_out)
    res = None
    try:
        if _RUNNER is None:
            _RUNNER = _build_runner(nc)
        parts = _RUNNER(in_maps)
    except Exception:
        from concourse.bass_utils import run_bass_kernel_spmd
        res = run_bass_kernel_spmd(nc, in_maps, list(range(8)), trace=trace)
        parts = [res.results[i]["out"] for i in range(8)]
    out = np.stack([parts[2 * b] + parts[2 * b + 1] for b in range(B)], axis=0)
    return out, res


def kernel(x, gamma, w_qkv, w_out):
    out, _ = run(x, gamma, w_qkv, w_out, trace=False)
    return out



# revision 60
# speedup vs baseline: 1.1116x; 1.1116x over previous
"""Trainium2 Bass kernel for nn_Attention (RMSNorm + QKV + 16-head attention + out-proj).

Sharding: 8 cores = 4 batches x 2 head-groups (DP x TP). Each core gets one
batch element and 8 of the 16 heads, computes a partial out-projection
([2048, 1024]); the host sums the two head-group partials per batch.

Per-core pipeline (T=2048 tokens, D=1024; all matmul operands bf16 with fp32
PSUM accumulation; measured end-to-end error is ~6e-3 scale-relative):
  Front (fused per token tile, so PE streams matmuls back-to-back while
      ScalarE does the RMS stats and DVE the casts/copies): load x,
      RMS-normalize (gamma*sqrt(D)*dh^-0.5*log2e folded into the weights on
      the host), 8 PE-transposes into one PSUM bank then a single strided
      copy to feature-major xnT [128, 8fc, T]; then the v projection for
      that tile (ones column appended per head so the AV matmul, M=65, also
      produces the softmax denominator in row 64 for free); after each
      512-token chunk, the q/k projections for that chunk.
  P3  attention per (pair, 512-wide q chunk): S^T tiles [128 keys, 2x512]
      with the two heads row-packed on the PE (K=64 at array rows 0-63 /
      64-127); one ScalarE exp over both banks (exp(ln2*x)=2^x via the free
      affine since log2e is folded into wq); AV accumulates per head into
      separate banks; 1/denom is broadcast across partitions with a tiny
      f32r ones-matmul; normalization tails are emitted one group late so
      their PE work never head-of-line-blocks the S stream; head-1 results
      are DMA-shifted to partitions 64:127 for the out-projection pair tile.
  P4  out-projection: matmul(lhsT=attn pair tile, rhs=w_out rows),
      accumulated over the 4 pairs in PSUM.

Measured on HW (loop-delta): phase fusion took the kernel 619us -> 443us on
a quiet device. A DVE exp offload (Schraudolph 2^x), deeper PSUM buffering
variants, SBUF-side tails, and 2-group slot interleaving were all tried and
benchmarked slower or neutral on HW; knobs remain in _CFG.

Toolchain workarounds: sync waits are capped at 1 per instruction (excess
moved onto NoOps via a BIR JSON post-pass) because this walrus rejects
multi-wait encodings; fp32r is used only where precision matters (1/denom
broadcast); gpsimd custom ops and DMA partition-broadcast are unavailable.
"""

import json
import numpy as np

B, T, D = 4, 2048, 1024
HEADS, DH = 16, 64
NT = T // 128   # 16 token tiles
FC = D // 128   # 8 feature chunks
NPAIR = 4       # head pairs per core (8 heads)
QCN = 4         # q chunks of 512
KT = NT         # key tiles

_PROG = {}

# Tuning knobs (read at program-build time; _PROG cache key includes them).
_CFG = {
    "dve_kt": (),  # key tiles whose exp runs on DVE (offload hurt on HW)
    "pi_copy": 0,  # 1: copy bitcast p through a bf16 tile before the AV MM
    "av_lag": 2,   # slots by which DVE-slot AV matmuls are issued late
    "s_bufs": 2,   # ps_s PSUM double/triple buffering (2 banks each)
    "av_bufs": 2,  # ps_av buffering (2 banks each)
    "exp_fd": 1024,  # 512 = timing probe with half the ScalarE exp work
    "s_merge": 0,  # 1: zero-padded qT + single N=1024 S matmul (ISA-illegal)
    "p4_wide": 0,  # 1: P4 matmuls at N=1024 (ISA-illegal: out spans 2 banks)
    "no_s2": 0,      # probe: skip the second S matmul (head1 scores garbage)
    "av_single": 0,  # probe: single AV matmul per slot (head1 out garbage)
    "cheap_tail": 0,  # probe: minimal tail (no normalization mults)
    "pp_bufs": 8,    # p_pool depth
    "deep": 0,       # 1: deepen SBUF-side pools (bsb/stg/rcp/xp/acc/pst)
    "streams": 1,    # 2: interleave two attention groups slot-by-slot
    "p4_pair": 1,    # 1: P4 hf-inner loop so lhsT is reused (LDW dedup)
    "tail_bf16": 1,  # 1: bf16 1/denom broadcast (full-rate stream vs f32r)
}


def _cfg_key():
    return tuple(sorted((k, tuple(v) if isinstance(v, (list, tuple)) else v)
                        for k, v in _CFG.items()))

# ---------------------------------------------------------------------------
# BIR post-pass: this walrus build rejects >1 sync wait per instruction in
# some encodings; move excess waits onto NoOps inserted before the offender.
_MAX_WAITS = 2
# opcodes whose walrus encoding only fits one sync wait
_ONE_WAIT_OPS = ()


def _split_excess_waits(bir_json: bytes) -> bytes:
    d = json.loads(bir_json)
    changed = False
    for fn in d.get("functions", []):
        for blk in fn.get("blocks", []):
            new_insts = []
            for inst in blk.get("instructions", []):
                si = inst.get("sync_info") or {}
                waits = si.get("on_wait") or []
                _MAX_WAITS = 1
                if len(waits) > _MAX_WAITS:
                    changed = True
                    excess = waits[: len(waits) - _MAX_WAITS]
                    si["on_wait"] = waits[len(waits) - _MAX_WAITS:]
                    inst["sync_info"] = si
                    for k in range(0, len(excess), _MAX_WAITS):
                        new_insts.append({
                            "debug": inst.get("debug", 0),
                            "engine": inst["engine"],
                            "ins": [],
                            "name": f"{inst['name']}-wsplit{k}",
                            "opcode": "NoOp",
                            "outs": [],
                            "sync_info": {
                                "on_update": [],
                                "on_wait": excess[k : k + _MAX_WAITS],
                            },
                        })
                new_insts.append(inst)
            blk["instructions"] = new_insts
    if not changed:
        return bir_json
    return json.dumps(d).encode()


def _install_bir_patch():
    import concourse.bass2jax as bass2jax
    import concourse.bass_utils as bass_utils

    if getattr(bass2jax.compile_bir_kernel, "_is_waitsplit_patch", False):
        return
    orig = bass_utils.compile_bir_kernel

    def patched(bir_json, tmpdir, neff_name="file.neff"):
        return orig(_split_excess_waits(bir_json), tmpdir, neff_name)

    patched._is_waitsplit_patch = True
    bass2jax.compile_bir_kernel = patched
    bass_utils.compile_bir_kernel = patched


# ---------------------------------------------------------------------------


def _build_program(loop_n=1, phases="all"):
    from contextlib import ExitStack

    import concourse.bass as bass
    import concourse.mybir as mybir
    import concourse.tile as tile
    from concourse.masks import make_identity

    F32 = mybir.dt.float32
    F32R = mybir.dt.float32r
    BF16 = mybir.dt.bfloat16
    I16 = mybir.dt.int16
    AF = mybir.ActivationFunctionType
    OP = mybir.AluOpType

    # exp work split across engines: key tiles in DVE_KT use the DVE
    # Schraudolph 2^x; the rest use ScalarE exp (scale=ln2).
    DVE_KT = frozenset(_CFG["dve_kt"])
    _SCHRAUDOLPH_BIAS = float(127 * 128 - 5.6)
    _LN2 = float(np.log(2.0))

    nc = bass.Bass("TRN2", target_bir_lowering=False, debug=False, num_devices=8)
    x_d = nc.dram_tensor("x", [T, D], F32, kind="ExternalInput").ap()
    wq_d = nc.dram_tensor("wq", [128, FC, 512], BF16, kind="ExternalInput").ap()
    wk_d = nc.dram_tensor("wk", [128, FC, 512], BF16, kind="ExternalInput").ap()
    wv_d = nc.dram_tensor("wv", [128, FC, 512], BF16, kind="ExternalInput").ap()
    wo_d = nc.dram_tensor("wo", [128, NPAIR, D], BF16, kind="ExternalInput").ap()
    out_d = nc.dram_tensor("out", [T, D], F32, kind="ExternalOutput").ap()

    with tile.TileContext(nc) as tc:
        with ExitStack() as es:
            singles = es.enter_context(tc.tile_pool(name="singles", bufs=1))
            qpool = es.enter_context(tc.tile_pool(name="qp", bufs=1))
            kpool = es.enter_context(tc.tile_pool(name="kp", bufs=1))
            vpool = es.enter_context(tc.tile_pool(name="vp", bufs=1))

            ident = singles.tile([128, 128], F32)
            make_identity(nc, ident[:])
            ident_bf = singles.tile([128, 128], BF16)
            nc.vector.tensor_copy(ident_bf[:], ident[:])
            ones_f32 = singles.tile([128, 64], F32)
            nc.gpsimd.memset(ones_f32[:], 1.0)
            ones_r = singles.tile([128, 64], F32R)
            nc.vector.tensor_copy(ones_r[:], ones_f32[:])
            ones_b = singles.tile([128, 64], BF16)
            nc.vector.tensor_copy(ones_b[:], ones_f32[:])
            stats = singles.tile([128, 64], F32)
            sqscratch = singles.tile([128, D], F32)

            # zero-padded qT for the merged S matmul: head h's dims live in
            # rows h*64:(h+1)*64 of slot h with the other half zero, so one
            # K=128 N=1024 matmul computes both heads without mixing them.
            if _CFG["s_merge"]:
                qTp = qpool.tile([128, NPAIR, QCN, 2, 512], BF16)
                nc.vector.memset(qTp[:], 0.0)

            # v with a ones column appended per head: AV matmul with M=65
            # yields attn_out rows 0:64 plus the softmax denominator in row 64
            v_sb = vpool.tile([128, NT, NPAIR, 2, 65], BF16)
            nc.vector.tensor_copy(
                v_sb[:, :, :, :, 64:65],
                ones_f32[:, 0:1].broadcast_to([128, NT, NPAIR, 2, 1]))
            aopool = es.enter_context(tc.tile_pool(name="aout", bufs=1))
            attn_sb = aopool.tile([128, NPAIR, T], BF16)

            import contextlib
            loop_ctx = (tc.For_i(0, loop_n, 1) if loop_n > 1
                        else contextlib.nullcontext())
            with loop_ctx:
                # SBUF frame that is released before the attention phase
                xnt_es = es.enter_context(ExitStack())
                xnt_pool = xnt_es.enter_context(tc.tile_pool(name="xnt", bufs=1))
                xnT = xnt_pool.tile([128, FC, T], BF16)

                if not _CFG["s_merge"]:
                    qTf = qpool.tile([128, NPAIR, T], BF16)
                kTf = kpool.tile([128, NPAIR, T], BF16)

                # ---- Fused front: per token tile, RMS-normalize + transpose
                # (P1) then the v projection (P2a); after each 512-token chunk
                # completes, the q/k projections for that chunk (P2b). Keeps
                # PE streaming back-to-back while ACT does the RMS stats and
                # DVE the casts/copies.
                front_es = es.enter_context(ExitStack())
                ps_t = front_es.enter_context(
                    tc.tile_pool(name="ps_t", bufs=3 if _CFG["deep"] else 2,
                                 space="PSUM"))
                ps_acc = front_es.enter_context(
                    tc.tile_pool(name="ps_acc", bufs=4 if _CFG["deep"] else 3,
                                 space="PSUM"))
                wqkv = front_es.enter_context(tc.tile_pool(name="wqkv", bufs=1))
                xp = front_es.enter_context(
                    tc.tile_pool(name="xin", bufs=4 if _CFG["deep"] else 3))
                wv_sb = wqkv.tile([128, FC, 512], BF16)
                nc.sync.dma_start(wv_sb[:], wv_d[:])
                wq_sb = wqkv.tile([128, FC, 512], BF16)
                nc.sync.dma_start(wq_sb[:], wq_d[:])
                wk_sb = wqkv.tile([128, FC, 512], BF16)
                nc.sync.dma_start(wk_sb[:], wk_d[:])
                for tt in range(NT):
                    x_t = xp.tile([128, D], F32, tag="x")
                    nc.sync.dma_start(x_t[:], x_d[tt * 128:(tt + 1) * 128, :])
                    ss = stats[:, tt:tt + 1]
                    nc.scalar.activation(
                        sqscratch[:], x_t[:], AF.Square, accum_out=ss)
                    nrm = stats[:, 16 + tt:17 + tt]
                    nc.scalar.sqrt(nrm, ss)
                    nc.vector.tensor_scalar_max(nrm, nrm, 1e-12)
                    rinv = stats[:, 32 + tt:33 + tt]
                    nc.vector.reciprocal(rinv, nrm)
                    xn_b = xp.tile([128, D], BF16, tag="xb")
                    nc.vector.tensor_scalar_mul(xn_b[:], x_t[:], rinv)
                    pt = ps_t.tile([128, D], BF16, tag="t")
                    for fc in range(FC):
                        nc.tensor.transpose(
                            pt[:, fc * 128:(fc + 1) * 128],
                            xn_b[:, fc * 128:(fc + 1) * 128], ident_bf[:])
                    nc.vector.tensor_copy(
                        xnT[:, :, tt * 128:(tt + 1) * 128],
                        pt[:].rearrange("p (f c) -> p f c", f=FC))
                    # P2a: v for this token tile
                    pv = ps_acc.tile([128, 512], F32, tag="acc")
                    for fc in range(FC):
                        nc.tensor.matmul(
                            pv[:], xnT[:, fc, tt * 128:(tt + 1) * 128],
                            wv_sb[:, fc, :],
                            start=(fc == 0), stop=(fc == FC - 1))
                    nc.vector.tensor_copy(
                        v_sb[:, tt, :, :, 0:64],
                        pv[:].rearrange("p (pr h c) -> p pr h c", pr=NPAIR, h=2))
                    # P2b: q/k for the completed 512-token chunk
                    if tt % 4 == 3:
                        qc = tt // 4
                        cs = slice(qc * 512, (qc + 1) * 512)
                        for p in range(NPAIR):
                            pq = ps_acc.tile([128, 512], F32, tag="acc")
                            for fc in range(FC):
                                nc.tensor.matmul(
                                    pq[:], wq_sb[:, fc, p * 128:(p + 1) * 128],
                                    xnT[:, fc, cs],
                                    start=(fc == 0), stop=(fc == FC - 1))
                            if _CFG["s_merge"]:
                                nc.vector.tensor_copy(
                                    qTp[0:64, p, qc, 0, :], pq[0:64, :])
                                nc.scalar.copy(
                                    qTp[64:128, p, qc, 1, :], pq[64:128, :])
                            else:
                                nc.vector.tensor_copy(qTf[:, p, cs], pq[:])
                            pk = ps_acc.tile([128, 512], F32, tag="acc")
                            for fc in range(FC):
                                nc.tensor.matmul(
                                    pk[:], wk_sb[:, fc, p * 128:(p + 1) * 128],
                                    xnT[:, fc, cs],
                                    start=(fc == 0), stop=(fc == FC - 1))
                            nc.scalar.copy(kTf[:, p, cs], pk[:])
                front_es.close()
                xnt_es.close()

                # ---- P3: attention; AV double-buffered, normalization tails
                # lagged one group so their PE work never blocks the S stream
                att_es = es.enter_context(ExitStack())
                ps_s = att_es.enter_context(
                    tc.tile_pool(name="ps_s", bufs=_CFG["s_bufs"], space="PSUM"))
                ps_av = att_es.enter_context(
                    tc.tile_pool(name="ps_av", bufs=_CFG["av_bufs"],
                                 space="PSUM"))
                p_pool = att_es.enter_context(
                    tc.tile_pool(name="pp", bufs=_CFG["pp_bufs"]))
                pi_pool = att_es.enter_context(tc.tile_pool(name="pip", bufs=3))
                av_pool = att_es.enter_context(tc.tile_pool(name="avs", bufs=2))
                _dp = 4 if _CFG["deep"] else 2
                rcp_pool = att_es.enter_context(
                    tc.tile_pool(name="rcp", bufs=_dp))
                bsb_pool = att_es.enter_context(
                    tc.tile_pool(name="bsb", bufs=_dp))
                stg_pool = att_es.enter_context(
                    tc.tile_pool(name="stg", bufs=_dp))

                def emit_rcp(pAV):
                    if _CFG["cheap_tail"]:
                        return None
                    # reciprocal of the denominator row; issued as soon as the
                    # AV group stops so the lagged pB matmuls never wait on DVE
                    rdt = BF16 if _CFG["tail_bf16"] else F32R
                    rcp = rcp_pool.tile([65, 1024], rdt, tag="rcp")
                    with nc.allow_low_precision(reason="1/denom feeds f32r matmul"):
                        nc.vector.reciprocal(rcp[64:65, :], pAV[64:65, :])
                    return rcp

                def emit_tail(p, qc, pAV, rcp):
                    cs = slice(qc * 512, (qc + 1) * 512)
                    if _CFG["cheap_tail"]:
                        # probe: unnormalized single-copy tail
                        nc.scalar.copy(attn_sb[0:64, p, cs], pAV[0:64, 0:512])
                        return
                    ones_t = ones_b if _CFG["tail_bf16"] else ones_r
                    pBa = ps_s.tile([128, 1024], F32, tag="s")
                    nc.tensor.matmul(
                        pBa[0:64, 0:512], ones_t[64:65, :], rcp[64:65, 0:512],
                        start=True, stop=True, tile_position=(64, 0))
                    nc.tensor.matmul(
                        pBa[0:64, 512:1024], ones_t[64:65, :], rcp[64:65, 512:1024],
                        start=True, stop=True, tile_position=(64, 0))
                    bsb = bsb_pool.tile([64, 1024], F32, tag="b")
                    nc.vector.tensor_copy(bsb[:], pBa[0:64, :])
                    nc.vector.tensor_tensor(
                        attn_sb[0:64, p, cs], pAV[0:64, 0:512], bsb[:, 0:512],
                        OP.mult)
                    stg = stg_pool.tile([64, 512], BF16, tag="stg")
                    nc.vector.tensor_tensor(
                        stg[:], pAV[0:64, 512:1024], bsb[:, 512:1024], OP.mult)
                    nc.sync.dma_start(attn_sb[64:128, p, cs], stg[:])

                # Precompute the AV issue schedule: DVE-slot AVs are issued
                # av_lag slots late so the in-order PE stream never
                # head-of-line-blocks on the DVE exp. (PSUM accumulation is
                # order-independent; start/stop go on the first/last ISSUED.)
                AV_LAG = _CFG["av_lag"]
                issue_seq = []   # kt values in AV issue order
                lag_sched = []   # per slot: list of lagged kt issued there
                _q = []
                for kt in range(KT):
                    due = [l for l in _q if l[1] <= kt]
                    _q = [l for l in _q if l[1] > kt]
                    here = [l[0] for l in due]
                    if kt in DVE_KT:
                        _q.append((kt, kt + AV_LAG))
                    else:
                        here.append(kt)
                    lag_sched.append(here)
                    issue_seq.extend(here)
                tail_flush = [l[0] for l in _q]
                issue_seq.extend(tail_flush)
                first_av, last_av = issue_seq[0], issue_seq[-1]

                if _CFG["streams"] == 2 and phases != "front":
                    # Two groups interleaved slot-by-slot: each group's
                    # S->exp->AV chain gets a full slot-pair of slack while
                    # the other group keeps the engines fed. Same 8 PSUM
                    # banks (each group single-buffers S and AV).
                    groups = [(p, qc) for p in range(NPAIR)
                              for qc in range(QCN)]
                    pend = []

                    def s2_av(c, kt, start, stop):
                        p, qc = c["p"], c["qc"]
                        p_t = c.pop("pt_" + str(kt))
                        nc.tensor.matmul(
                            c["pAV"][0:65, 0:512],
                            v_sb[:, kt, p, 0, :], p_t[:, 0:512],
                            start=start, stop=stop)
                        nc.tensor.matmul(
                            c["pAV"][0:65, 512:1024],
                            v_sb[:, kt, p, 1, :], p_t[:, 512:1024],
                            start=start, stop=stop)

                    for blk in range(0, len(groups), 2):
                        ctxs = []
                        for (p, qc) in groups[blk:blk + 2]:
                            pAV = ps_av.tile(
                                [128, 1024], F32, tag="av", name="pAV")
                            ctxs.append({"p": p, "qc": qc, "pAV": pAV})
                        for kt in range(KT):
                            for c in ctxs:
                                p, qc = c["p"], c["qc"]
                                cs = slice(qc * 512, (qc + 1) * 512)
                                ks = slice(kt * 128, (kt + 1) * 128)
                                kT = kTf[:, p, :]
                                qT = qTf[:, p, :]
                                pS = ps_s.tile([128, 1024], F32, tag="s")
                                nc.tensor.matmul(
                                    pS[:, 0:512], kT[0:64, ks], qT[0:64, cs],
                                    start=True, stop=True,
                                    tile_position=(0, 0))
                                nc.tensor.matmul(
                                    pS[:, 512:1024], kT[64:128, ks],
                                    qT[64:128, cs],
                                    start=True, stop=True,
                                    tile_position=(64, 0))
                                c["pS"] = pS
                            if kt == 0 and pend:
                                emit_tail(*pend.pop(0))
                            if kt == 1 and pend:
                                emit_tail(*pend.pop(0))
                            for c in ctxs:
                                if kt > 0:
                                    s2_av(c, kt - 1, kt - 1 == 0, False)
                            for c in ctxs:
                                p_tile = p_pool.tile(
                                    [128, 1024], BF16, tag="P")
                                nc.scalar.activation(
                                    p_tile[:], c["pS"], AF.Exp, scale=_LN2)
                                c["pt_" + str(kt)] = p_tile[:]
                        for c in ctxs:
                            s2_av(c, KT - 1, False, True)
                        for c in ctxs:
                            rcp = emit_rcp(c["pAV"])
                            pend.append((c["p"], c["qc"], c["pAV"], rcp))
                    for t in pend:
                        emit_tail(*t)

                pending = None
                for p in range(NPAIR if (phases != "front"
                                         and _CFG["streams"] == 1) else 0):
                    qT = None if _CFG["s_merge"] else qTf[:, p, :]
                    kT = kTf[:, p, :]
                    for qc in range(QCN):
                        cs = slice(qc * 512, (qc + 1) * 512)
                        pAV = ps_av.tile([128, 1024], F32, tag="av")
                        p_aps = {}

                        def emit_av(kt):
                            st = kt == first_av
                            sp = kt == last_av
                            p0, p1 = p_aps.pop(kt)
                            nc.tensor.matmul(
                                pAV[0:65, 0:512],
                                v_sb[:, kt, p, 0, :], p0,
                                start=st, stop=sp)
                            if _CFG["av_single"]:
                                return
                            nc.tensor.matmul(
                                pAV[0:65, 512:1024],
                                v_sb[:, kt, p, 1, :], p1,
                                start=st, stop=sp)

                        for kt in range(KT):
                            ks = slice(kt * 128, (kt + 1) * 128)
                            pS = ps_s.tile([128, 1024], F32, tag="s")
                            if _CFG["no_s2"]:
                                nc.tensor.matmul(
                                    pS[:, 0:512], kT[0:64, ks], qT[0:64, cs],
                                    start=True, stop=True,
                                    tile_position=(0, 0))
                            elif _CFG["s_merge"]:
                                # one K=128 N=1024 matmul; the zero padding in
                                # qTp keeps the heads separate
                                nc.tensor.matmul(
                                    pS[:], kT[:, ks],
                                    qTp[:, p, qc, :, :].rearrange(
                                        "p a b -> p (a b)"),
                                    start=True, stop=True)
                            else:
                                nc.tensor.matmul(
                                    pS[:, 0:512], kT[0:64, ks], qT[0:64, cs],
                                    start=True, stop=True, tile_position=(0, 0))
                                nc.tensor.matmul(
                                    pS[:, 512:1024], kT[64:128, ks],
                                    qT[64:128, cs],
                                    start=True, stop=True,
                                    tile_position=(64, 0))
                            # S is in log2 domain (log2e folded into wq).
                            # ScalarE slots: exp(ln2*x) = 2^x via the free
                            # affine. DVE slots: Schraudolph 2^x -- int16
                            # round(128x + 16256 - C) bitcast to bf16 (max
                            # ~3.3% exp err on ~1/3 of key blocks; softmax
                            # denominators stay consistent via the ones
                            # column in v).
                            if kt in DVE_KT:
                                pi = pi_pool.tile([128, 1024], I16, tag="Pi")
                                nc.vector.tensor_scalar(
                                    pi[:], pS[:], 128.0, _SCHRAUDOLPH_BIAS,
                                    OP.mult, OP.add)
                                if _CFG["pi_copy"]:
                                    p_tile = p_pool.tile(
                                        [128, 1024], BF16, tag="P")
                                    nc.vector.tensor_copy(
                                        p_tile[:], pi[:].bitcast(BF16))
                                    pb = p_tile[:]
                                else:
                                    pb = pi[:].bitcast(BF16)
                                p_aps[kt] = (pb[:, 0:512], pb[:, 512:1024])
                            elif _CFG["exp_fd"] == 512:
                                # timing probe: half the ScalarE exp work
                                # (math wrong; head1 reuses head0's p)
                                p_tile = p_pool.tile([128, 1024], BF16, tag="P")
                                nc.scalar.activation(
                                    p_tile[:, 0:512], pS[:, 0:512],
                                    AF.Exp, scale=_LN2)
                                p_aps[kt] = (p_tile[:, 0:512],
                                             p_tile[:, 0:512])
                            else:
                                p_tile = p_pool.tile([128, 1024], BF16, tag="P")
                                nc.scalar.activation(
                                    p_tile[:], pS[:], AF.Exp, scale=_LN2)
                                p_aps[kt] = (p_tile[:, 0:512],
                                             p_tile[:, 512:1024])
                            # AV with ones-augmented v: rows 0:64 attn out,
                            # row 64 the softmax denominator (bank per head)
                            for lkt in lag_sched[kt]:
                                emit_av(lkt)
                            if kt == 4 and pending is not None:
                                emit_tail(*pending)
                                pending = None
                        for lkt in tail_flush:
                            emit_av(lkt)
                        rcp = emit_rcp(pAV)
                        pending = (p, qc, pAV, rcp)
                if pending is not None:
                    emit_tail(*pending)
                att_es.close()

                # ---- P4: out projection (accumulate over the 4 pairs)
                with tc.tile_pool(name="ps_o", bufs=3, space="PSUM") as ps_o, \
                     tc.tile_pool(name="wop", bufs=1) as wop, \
                     tc.tile_pool(name="osb", bufs=3) as osb:
                    wo_sb = wop.tile([128, NPAIR, D], BF16)
                    nc.sync.dma_start(wo_sb[:], wo_d[:])
                    for tt in range(NT):
                        if phases == "front":
                            # consume q/k/v so the front isn't dead code
                            qsrc = (qTp[:, tt % NPAIR, 0, 0, :]
                                    if _CFG["s_merge"] else
                                    qTf[:, tt % NPAIR, 0:512])
                            dm = osb.tile([128, 512], F32, tag="o")
                            nc.vector.tensor_tensor(
                                dm[:], qsrc,
                                kTf[:, tt % NPAIR, 0:512], OP.mult)
                            nc.vector.tensor_tensor(
                                dm[:, 0:65], v_sb[:, tt, 0, 0, :], dm[:, 0:65],
                                OP.mult)
                            nc.sync.dma_start(
                                out_d[tt * 128:(tt + 1) * 128, 0:512], dm[:])
                            continue
                        if phases != "all":
                            # consume attn_sb so attention isn't dead code
                            dm = osb.tile([128, 512], F32, tag="o")
                            nc.vector.tensor_copy(
                                dm[:], attn_sb[:, tt % NPAIR, 0:512])
                            nc.sync.dma_start(
                                out_d[tt * 128:(tt + 1) * 128, 0:512], dm[:])
                            continue
                        if _CFG["p4_pair"]:
                            # hf-inner: consecutive matmuls share lhsT (the
                            # attn pair tile) so the legalizer drops half the
                            # Ldweights
                            po0 = ps_o.tile([128, 512], F32, tag="o", name="po0")
                            po1 = ps_o.tile([128, 512], F32, tag="o", name="po1")
                            for p in range(NPAIR):
                                at = attn_sb[:, p, tt * 128:(tt + 1) * 128]
                                nc.tensor.matmul(
                                    po0[:], at, wo_sb[:, p, 0:512],
                                    start=(p == 0), stop=(p == NPAIR - 1))
                                nc.tensor.matmul(
                                    po1[:], at, wo_sb[:, p, 512:1024],
                                    start=(p == 0), stop=(p == NPAIR - 1))
                            for hf, po in ((0, po0), (1, po1)):
                                o_sb = osb.tile([128, 512], F32, tag="o")
                                if hf == 0:
                                    nc.vector.tensor_copy(o_sb[:], po[:])
                                else:
                                    nc.scalar.copy(o_sb[:], po[:])
                                nc.sync.dma_start(
                                    out_d[tt * 128:(tt + 1) * 128,
                                          hf * 512:(hf + 1) * 512], o_sb[:])
                            continue
                        if _CFG["p4_wide"]:
                            po = ps_o.tile([128, 1024], F32, tag="ow")
                            for p in range(NPAIR):
                                nc.tensor.matmul(
                                    po[:], attn_sb[:, p, tt * 128:(tt + 1) * 128],
                                    wo_sb[:, p, :],
                                    start=(p == 0), stop=(p == NPAIR - 1))
                            o_sb = osb.tile([128, 1024], F32, tag="ow")
                            if tt % 2 == 0:
                                nc.vector.tensor_copy(o_sb[:], po[:])
                            else:
                                nc.scalar.copy(o_sb[:], po[:])
                            nc.sync.dma_start(
                                out_d[tt * 128:(tt + 1) * 128, :], o_sb[:])
                            continue
                        for hf in range(2):
                            po = ps_o.tile([128, 512], F32, tag="o")
                            for p in range(NPAIR):
                                nc.tensor.matmul(
                                    po[:], attn_sb[:, p, tt * 128:(tt + 1) * 128],
                                    wo_sb[:, p, hf * 512:(hf + 1) * 512],
                                    start=(p == 0), stop=(p == NPAIR - 1))
                            o_sb = osb.tile([128, 512], F32, tag="o")
                            nc.vector.tensor_copy(o_sb[:], po[:])
                            nc.sync.dma_start(
                                out_d[tt * 128:(tt + 1) * 128, hf * 512:(hf + 1) * 512],
                                o_sb[:])
    return nc


def _get_program(loop_n=1, phases="all"):
    key = (loop_n, phases, _cfg_key())
    if key not in _PROG:
        _install_bir_patch()
        _PROG[key] = _build_program(loop_n, phases)
    return _PROG[key]


def _make_in_maps(x, gamma, w_qkv, w_out):
    x = np.asarray(x, dtype=np.float32)
    gamma = np.asarray(gamma, dtype=np.float32)
    w_qkv = np.asarray(w_qkv, dtype=np.float32)
    w_out = np.asarray(w_out, dtype=np.float32)

    scale = gamma * np.float32(np.sqrt(D))          # fold sqrt(D)*gamma
    in_maps = []
    for core in range(8):
        b = core // 2
        hg = core % 2
        cols = slice(hg * 512, (hg + 1) * 512)
        wq = (w_qkv[:, 0 * D:1 * D][:, cols] * scale[:, None]
              * np.float32(DH ** -0.5) * np.float32(np.log2(np.e)))
        wk = w_qkv[:, 1 * D:2 * D][:, cols] * scale[:, None]
        wv = w_qkv[:, 2 * D:3 * D][:, cols] * scale[:, None]
        wo = w_out[hg * 512:(hg + 1) * 512, :]
        import ml_dtypes
        bf16 = ml_dtypes.bfloat16
        in_maps.append({
            "x": np.ascontiguousarray(x[b]),
            "wq": np.ascontiguousarray(
                wq.reshape(FC, 128, 512).transpose(1, 0, 2).astype(bf16)),
            "wk": np.ascontiguousarray(
                wk.reshape(FC, 128, 512).transpose(1, 0, 2).astype(bf16)),
            "wv": np.ascontiguousarray(
                wv.reshape(FC, 128, 512).transpose(1, 0, 2).astype(bf16)),
            "wo": np.ascontiguousarray(
                wo.reshape(NPAIR, 128, D).transpose(1, 0, 2).astype(bf16)),
        })
    return in_maps


_RUNNER = None


def _build_runner(nc):
    """Persistent jitted callable over the 8-core mesh (avoids per-call
    re-tracing that run_bass_kernel_spmd incurs)."""
    import jax
    import concourse.mybir as mybir
    from jax.sharding import Mesh, PartitionSpec
    from jax.experimental.shard_map import shard_map
    from concourse.bass2jax import (
        _bass_exec_p, install_neuronx_cc_hook, partition_id_tensor)

    install_neuronx_cc_hook()
    partition_name = nc.partition_id_tensor.name if nc.partition_id_tensor else None
    in_names, out_names, out_avals, zero_shapes = [], [], [], []
    for alloc in nc.m.functions[0].allocations:
        if not isinstance(alloc, mybir.MemoryLocationSet):
            continue
        name = alloc.memorylocations[0].name
        if alloc.kind == "ExternalInput":
            if name != partition_name:
                in_names.append(name)
        elif alloc.kind == "ExternalOutput":
            out_names.append(name)
            shape = tuple(alloc.tensor_shape)
            dtype = mybir.dt.np(alloc.dtype)
            out_avals.append(jax.core.ShapedArray(shape, dtype))
            zero_shapes.append((shape, dtype))
    n_params = len(in_names)
    all_in_names = tuple(in_names + out_names)
    if partition_name is not None:
        all_in_names = all_in_names + (partition_name,)

    def _body(*args):
        operands = list(args)
        if partition_name is not None:
            operands.append(partition_id_tensor())
        return tuple(_bass_exec_p.bind(
            *operands,
            out_avals=tuple(out_avals),
            in_names=all_in_names,
            out_names=tuple(out_names),
            lowering_input_output_aliases=(),
            sim_require_finite=True,
            sim_require_nnan=True,
            nc=nc,
        ))

    devices = jax.devices()[:8]
    mesh = Mesh(np.asarray(devices), ("core",))
    nin = n_params + len(out_names)
    fn = jax.jit(shard_map(
        _body, mesh=mesh, in_specs=(PartitionSpec("core"),) * nin,
        out_specs=(PartitionSpec("core"),) * len(out_names), check_rep=False))

    def runner(in_maps):
        args = [np.concatenate([np.asarray(in_maps[c][nm]) for c in range(8)],
                               axis=0) for nm in in_names]
        for shape, dtype in zero_shapes:
            args.append(np.zeros((8 * shape[0], *shape[1:]), dtype))
        outs = fn(*args)
        o = np.asarray(outs[0]).reshape(8, T, D)
        return [o[c] for c in range(8)]

    return runner


def run(x, gamma, w_qkv, w_out, trace=False):
    """Run on the 8 NeuronCores; returns (output, results-or-None)."""
    global _RUNNER
    nc = _get_program()
    in_maps = _make_in_maps(x, gamma, w_qkv, w_out)
    res = None
    try:
        if _RUNNER is None:
            _RUNNER = _build_runner(nc)
        parts = _RUNNER(in_maps)
    except Exception:
        from concourse.bass_utils import run_bass_kernel_spmd
        res = run_bass_kernel_spmd(nc, in_maps, list(range(8)), trace=trace)
        parts = [res.results[i]["out"] for i in range(8)]
    out = np.stack([parts[2 * b] + parts[2 * b + 1] for b in range(B)], axis=0)
    return out, res


def kernel(x, gamma, w_qkv, w_out):
    out, _ = run(x, gamma, w_qkv, w_out, trace=False)
    return out



# revision 64
# speedup vs baseline: 1.9525x; 1.7564x over previous
"""Trainium2 Bass kernel for nn_Attention (RMSNorm + QKV + 16-head attention + out-proj).

Sharding: 8 cores = 4 batches x 2 head-groups (DP x TP). Each core gets one
batch element and 8 of the 16 heads, computes a partial out-projection
([2048, 1024]); the host sums the two head-group partials per batch.

Per-core pipeline (T=2048 tokens, D=1024; all matmul operands bf16 with fp32
PSUM accumulation; measured end-to-end error is ~6e-3 scale-relative):
  Front (fused per token tile, so PE streams matmuls back-to-back while
      ScalarE does the RMS stats and DVE the casts/copies): load x,
      RMS-normalize (gamma*sqrt(D)*dh^-0.5*log2e folded into the weights on
      the host), 8 PE-transposes into one PSUM bank then a single strided
      copy to feature-major xnT [128, 8fc, T]; then the v projection for
      that tile (ones column appended per head so the AV matmul, M=65, also
      produces the softmax denominator in row 64 for free); after each
      512-token chunk, the q/k projections for that chunk.
  P3  attention per (pair, 512-wide q chunk): S^T tiles [128 keys, 2x512]
      with the two heads row-packed on the PE (K=64 at array rows 0-63 /
      64-127); one ScalarE exp over both banks (exp(ln2*x)=2^x via the free
      affine since log2e is folded into wq); AV accumulates per head into
      separate banks; 1/denom is broadcast across partitions with a tiny
      f32r ones-matmul; normalization tails are emitted one group late so
      their PE work never head-of-line-blocks the S stream; head-1 results
      are DMA-shifted to partitions 64:127 for the out-projection pair tile.
  P4  out-projection: matmul(lhsT=attn pair tile, rhs=w_out rows),
      accumulated over the 4 pairs in PSUM.

Measured on HW (loop-delta): phase fusion took the kernel 619us -> 443us on
a quiet device. A DVE exp offload (Schraudolph 2^x), deeper PSUM buffering
variants, SBUF-side tails, and 2-group slot interleaving were all tried and
benchmarked slower or neutral on HW; knobs remain in _CFG.

Toolchain workarounds: sync waits are capped at 1 per instruction (excess
moved onto NoOps via a BIR JSON post-pass) because this walrus rejects
multi-wait encodings; fp32r is used only where precision matters (1/denom
broadcast); gpsimd custom ops and DMA partition-broadcast are unavailable.
"""

import json
import numpy as np

B, T, D = 4, 2048, 1024
HEADS, DH = 16, 64
NT = T // 128   # 16 token tiles
FC = D // 128   # 8 feature chunks
NPAIR = 4       # head pairs per core (8 heads)
QCN = 4         # q chunks of 512
KT = NT         # key tiles

_PROG = {}

# Tuning knobs (read at program-build time; _PROG cache key includes them).
_CFG = {
    "dve_kt": (),  # key tiles whose exp runs on DVE (offload hurt on HW)
    "pi_copy": 0,  # 1: copy bitcast p through a bf16 tile before the AV MM
    "av_lag": 1,   # slots by which lagged AV matmuls are issued late
    "s_bufs": 2,   # ps_s PSUM double/triple buffering (2 banks each)
    "av_bufs": 2,  # ps_av buffering (2 banks each)
    "exp_fd": 1024,  # 512 = timing probe with half the ScalarE exp work
    "s_merge": 0,  # 1: zero-padded qT + single N=1024 S matmul (ISA-illegal)
    "p4_wide": 0,  # 1: P4 matmuls at N=1024 (ISA-illegal: out spans 2 banks)
    "no_s2": 0,      # probe: skip the second S matmul (head1 scores garbage)
    "av_single": 0,  # probe: single AV matmul per slot (head1 out garbage)
    "cheap_tail": 0,  # probe: minimal tail (no normalization mults)
    "pp_bufs": 8,    # p_pool depth
    "deep": 0,       # 1: deepen SBUF-side pools (bsb/stg/rcp/xp/acc/pst)
    "streams": 1,    # 2: interleave two attention groups slot-by-slot
    "p4_pair": 1,    # 1: P4 hf-inner loop so lhsT is reused (LDW dedup)
    "tail_bf16": 1,  # 1: bf16 1/denom broadcast (full-rate stream vs f32r)
    "lag_all": 1,    # 1: lag ALL AV matmuls by av_lag slots (exp runs ahead
                     #    of the PE so AV never head-of-line-blocks on ACT)
}


def _cfg_key():
    return tuple(sorted((k, tuple(v) if isinstance(v, (list, tuple)) else v)
                        for k, v in _CFG.items()))

# ---------------------------------------------------------------------------
# BIR post-pass: this walrus build rejects >1 sync wait per instruction in
# some encodings; move excess waits onto NoOps inserted before the offender.
_MAX_WAITS = 2
# opcodes whose walrus encoding only fits one sync wait
_ONE_WAIT_OPS = ()


def _split_excess_waits(bir_json: bytes) -> bytes:
    d = json.loads(bir_json)
    changed = False
    for fn in d.get("functions", []):
        for blk in fn.get("blocks", []):
            new_insts = []
            for inst in blk.get("instructions", []):
                si = inst.get("sync_info") or {}
                waits = si.get("on_wait") or []
                _MAX_WAITS = 1
                if len(waits) > _MAX_WAITS:
                    changed = True
                    excess = waits[: len(waits) - _MAX_WAITS]
                    si["on_wait"] = waits[len(waits) - _MAX_WAITS:]
                    inst["sync_info"] = si
                    for k in range(0, len(excess), _MAX_WAITS):
                        new_insts.append({
                            "debug": inst.get("debug", 0),
                            "engine": inst["engine"],
                            "ins": [],
                            "name": f"{inst['name']}-wsplit{k}",
                            "opcode": "NoOp",
                            "outs": [],
                            "sync_info": {
                                "on_update": [],
                                "on_wait": excess[k : k + _MAX_WAITS],
                            },
                        })
                new_insts.append(inst)
            blk["instructions"] = new_insts
    if not changed:
        return bir_json
    return json.dumps(d).encode()


def _install_bir_patch():
    import concourse.bass2jax as bass2jax
    import concourse.bass_utils as bass_utils

    if getattr(bass2jax.compile_bir_kernel, "_is_waitsplit_patch", False):
        return
    orig = bass_utils.compile_bir_kernel

    def patched(bir_json, tmpdir, neff_name="file.neff"):
        return orig(_split_excess_waits(bir_json), tmpdir, neff_name)

    patched._is_waitsplit_patch = True
    bass2jax.compile_bir_kernel = patched
    bass_utils.compile_bir_kernel = patched


# ---------------------------------------------------------------------------


def _build_program(loop_n=1, phases="all"):
    from contextlib import ExitStack

    import concourse.bass as bass
    import concourse.mybir as mybir
    import concourse.tile as tile
    from concourse.masks import make_identity

    F32 = mybir.dt.float32
    F32R = mybir.dt.float32r
    BF16 = mybir.dt.bfloat16
    I16 = mybir.dt.int16
    AF = mybir.ActivationFunctionType
    OP = mybir.AluOpType

    # exp work split across engines: key tiles in DVE_KT use the DVE
    # Schraudolph 2^x; the rest use ScalarE exp (scale=ln2).
    DVE_KT = frozenset(_CFG["dve_kt"])
    _SCHRAUDOLPH_BIAS = float(127 * 128 - 5.6)
    _LN2 = float(np.log(2.0))

    nc = bass.Bass("TRN2", target_bir_lowering=False, debug=False, num_devices=8)
    x_d = nc.dram_tensor("x", [T, D], F32, kind="ExternalInput").ap()
    wq_d = nc.dram_tensor("wq", [128, FC, 512], BF16, kind="ExternalInput").ap()
    wk_d = nc.dram_tensor("wk", [128, FC, 512], BF16, kind="ExternalInput").ap()
    wv_d = nc.dram_tensor("wv", [128, FC, 512], BF16, kind="ExternalInput").ap()
    wo_d = nc.dram_tensor("wo", [128, NPAIR, D], BF16, kind="ExternalInput").ap()
    out_d = nc.dram_tensor("out", [T, D], F32, kind="ExternalOutput").ap()

    with tile.TileContext(nc) as tc:
        with ExitStack() as es:
            singles = es.enter_context(tc.tile_pool(name="singles", bufs=1))
            qpool = es.enter_context(tc.tile_pool(name="qp", bufs=1))
            kpool = es.enter_context(tc.tile_pool(name="kp", bufs=1))
            vpool = es.enter_context(tc.tile_pool(name="vp", bufs=1))

            ident = singles.tile([128, 128], F32)
            make_identity(nc, ident[:])
            ident_bf = singles.tile([128, 128], BF16)
            nc.vector.tensor_copy(ident_bf[:], ident[:])
            ones_f32 = singles.tile([128, 64], F32)
            nc.gpsimd.memset(ones_f32[:], 1.0)
            ones_r = singles.tile([128, 64], F32R)
            nc.vector.tensor_copy(ones_r[:], ones_f32[:])
            ones_b = singles.tile([128, 64], BF16)
            nc.vector.tensor_copy(ones_b[:], ones_f32[:])
            stats = singles.tile([128, 64], F32)
            sqscratch = singles.tile([128, D], F32)

            # zero-padded qT for the merged S matmul: head h's dims live in
            # rows h*64:(h+1)*64 of slot h with the other half zero, so one
            # K=128 N=1024 matmul computes both heads without mixing them.
            if _CFG["s_merge"]:
                qTp = qpool.tile([128, NPAIR, QCN, 2, 512], BF16)
                nc.vector.memset(qTp[:], 0.0)

            # v with a ones column appended per head: AV matmul with M=65
            # yields attn_out rows 0:64 plus the softmax denominator in row 64
            v_sb = vpool.tile([128, NT, NPAIR, 2, 65], BF16)
            nc.vector.tensor_copy(
                v_sb[:, :, :, :, 64:65],
                ones_f32[:, 0:1].broadcast_to([128, NT, NPAIR, 2, 1]))
            aopool = es.enter_context(tc.tile_pool(name="aout", bufs=1))
            attn_sb = aopool.tile([128, NPAIR, T], BF16)

            import contextlib
            loop_ctx = (tc.For_i(0, loop_n, 1) if loop_n > 1
                        else contextlib.nullcontext())
            with loop_ctx:
                # SBUF frame that is released before the attention phase
                xnt_es = es.enter_context(ExitStack())
                xnt_pool = xnt_es.enter_context(tc.tile_pool(name="xnt", bufs=1))
                xnT = xnt_pool.tile([128, FC, T], BF16)

                if not _CFG["s_merge"]:
                    qTf = qpool.tile([128, NPAIR, T], BF16)
                kTf = kpool.tile([128, NPAIR, T], BF16)

                # ---- Fused front: per token tile, RMS-normalize + transpose
                # (P1) then the v projection (P2a); after each 512-token chunk
                # completes, the q/k projections for that chunk (P2b). Keeps
                # PE streaming back-to-back while ACT does the RMS stats and
                # DVE the casts/copies.
                front_es = es.enter_context(ExitStack())
                ps_t = front_es.enter_context(
                    tc.tile_pool(name="ps_t", bufs=3 if _CFG["deep"] else 2,
                                 space="PSUM"))
                ps_acc = front_es.enter_context(
                    tc.tile_pool(name="ps_acc", bufs=4 if _CFG["deep"] else 3,
                                 space="PSUM"))
                wqkv = front_es.enter_context(tc.tile_pool(name="wqkv", bufs=1))
                xp = front_es.enter_context(
                    tc.tile_pool(name="xin", bufs=4 if _CFG["deep"] else 3))
                wv_sb = wqkv.tile([128, FC, 512], BF16)
                nc.sync.dma_start(wv_sb[:], wv_d[:])
                wq_sb = wqkv.tile([128, FC, 512], BF16)
                nc.sync.dma_start(wq_sb[:], wq_d[:])
                wk_sb = wqkv.tile([128, FC, 512], BF16)
                nc.sync.dma_start(wk_sb[:], wk_d[:])
                for tt in range(NT):
                    x_t = xp.tile([128, D], F32, tag="x")
                    nc.sync.dma_start(x_t[:], x_d[tt * 128:(tt + 1) * 128, :])
                    ss = stats[:, tt:tt + 1]
                    nc.scalar.activation(
                        sqscratch[:], x_t[:], AF.Square, accum_out=ss)
                    nrm = stats[:, 16 + tt:17 + tt]
                    nc.scalar.sqrt(nrm, ss)
                    nc.vector.tensor_scalar_max(nrm, nrm, 1e-12)
                    rinv = stats[:, 32 + tt:33 + tt]
                    nc.vector.reciprocal(rinv, nrm)
                    xn_b = xp.tile([128, D], BF16, tag="xb")
                    nc.vector.tensor_scalar_mul(xn_b[:], x_t[:], rinv)
                    pt = ps_t.tile([128, D], BF16, tag="t")
                    for fc in range(FC):
                        nc.tensor.transpose(
                            pt[:, fc * 128:(fc + 1) * 128],
                            xn_b[:, fc * 128:(fc + 1) * 128], ident_bf[:])
                    nc.vector.tensor_copy(
                        xnT[:, :, tt * 128:(tt + 1) * 128],
                        pt[:].rearrange("p (f c) -> p f c", f=FC))
                    # P2a: v for this token tile
                    pv = ps_acc.tile([128, 512], F32, tag="acc")
                    for fc in range(FC):
                        nc.tensor.matmul(
                            pv[:], xnT[:, fc, tt * 128:(tt + 1) * 128],
                            wv_sb[:, fc, :],
                            start=(fc == 0), stop=(fc == FC - 1))
                    nc.vector.tensor_copy(
                        v_sb[:, tt, :, :, 0:64],
                        pv[:].rearrange("p (pr h c) -> p pr h c", pr=NPAIR, h=2))
                    # P2b: q/k for the completed 512-token chunk
                    if tt % 4 == 3:
                        qc = tt // 4
                        cs = slice(qc * 512, (qc + 1) * 512)
                        for p in range(NPAIR):
                            pq = ps_acc.tile([128, 512], F32, tag="acc")
                            for fc in range(FC):
                                nc.tensor.matmul(
                                    pq[:], wq_sb[:, fc, p * 128:(p + 1) * 128],
                                    xnT[:, fc, cs],
                                    start=(fc == 0), stop=(fc == FC - 1))
                            if _CFG["s_merge"]:
                                nc.vector.tensor_copy(
                                    qTp[0:64, p, qc, 0, :], pq[0:64, :])
                                nc.scalar.copy(
                                    qTp[64:128, p, qc, 1, :], pq[64:128, :])
                            else:
                                nc.vector.tensor_copy(qTf[:, p, cs], pq[:])
                            pk = ps_acc.tile([128, 512], F32, tag="acc")
                            for fc in range(FC):
                                nc.tensor.matmul(
                                    pk[:], wk_sb[:, fc, p * 128:(p + 1) * 128],
                                    xnT[:, fc, cs],
                                    start=(fc == 0), stop=(fc == FC - 1))
                            nc.scalar.copy(kTf[:, p, cs], pk[:])
                front_es.close()
                xnt_es.close()

                # ---- P3: attention; AV double-buffered, normalization tails
                # lagged one group so their PE work never blocks the S stream
                att_es = es.enter_context(ExitStack())
                ps_s = att_es.enter_context(
                    tc.tile_pool(name="ps_s", bufs=_CFG["s_bufs"], space="PSUM"))
                ps_av = att_es.enter_context(
                    tc.tile_pool(name="ps_av", bufs=_CFG["av_bufs"],
                                 space="PSUM"))
                p_pool = att_es.enter_context(
                    tc.tile_pool(name="pp", bufs=_CFG["pp_bufs"]))
                pi_pool = att_es.enter_context(tc.tile_pool(name="pip", bufs=3))
                av_pool = att_es.enter_context(tc.tile_pool(name="avs", bufs=2))
                _dp = 4 if _CFG["deep"] else 2
                rcp_pool = att_es.enter_context(
                    tc.tile_pool(name="rcp", bufs=_dp))
                bsb_pool = att_es.enter_context(
                    tc.tile_pool(name="bsb", bufs=_dp))
                stg_pool = att_es.enter_context(
                    tc.tile_pool(name="stg", bufs=_dp))

                def emit_rcp(pAV):
                    if _CFG["cheap_tail"]:
                        return None
                    # reciprocal of the denominator row; issued as soon as the
                    # AV group stops so the lagged pB matmuls never wait on DVE
                    rdt = BF16 if _CFG["tail_bf16"] else F32R
                    rcp = rcp_pool.tile([65, 1024], rdt, tag="rcp")
                    with nc.allow_low_precision(reason="1/denom feeds f32r matmul"):
                        nc.vector.reciprocal(rcp[64:65, :], pAV[64:65, :])
                    return rcp

                def emit_tail(p, qc, pAV, rcp):
                    cs = slice(qc * 512, (qc + 1) * 512)
                    if _CFG["cheap_tail"]:
                        # probe: unnormalized single-copy tail
                        nc.scalar.copy(attn_sb[0:64, p, cs], pAV[0:64, 0:512])
                        return
                    ones_t = ones_b if _CFG["tail_bf16"] else ones_r
                    pBa = ps_s.tile([128, 1024], F32, tag="s")
                    nc.tensor.matmul(
                        pBa[0:64, 0:512], ones_t[64:65, :], rcp[64:65, 0:512],
                        start=True, stop=True, tile_position=(64, 0))
                    nc.tensor.matmul(
                        pBa[0:64, 512:1024], ones_t[64:65, :], rcp[64:65, 512:1024],
                        start=True, stop=True, tile_position=(64, 0))
                    bsb = bsb_pool.tile([64, 1024], F32, tag="b")
                    nc.vector.tensor_copy(bsb[:], pBa[0:64, :])
                    nc.vector.tensor_tensor(
                        attn_sb[0:64, p, cs], pAV[0:64, 0:512], bsb[:, 0:512],
                        OP.mult)
                    stg = stg_pool.tile([64, 512], BF16, tag="stg")
                    nc.vector.tensor_tensor(
                        stg[:], pAV[0:64, 512:1024], bsb[:, 512:1024], OP.mult)
                    nc.sync.dma_start(attn_sb[64:128, p, cs], stg[:])

                # Precompute the AV issue schedule: DVE-slot AVs are issued
                # av_lag slots late so the in-order PE stream never
                # head-of-line-blocks on the DVE exp. (PSUM accumulation is
                # order-independent; start/stop go on the first/last ISSUED.)
                AV_LAG = _CFG["av_lag"]
                issue_seq = []   # kt values in AV issue order
                lag_sched = []   # per slot: list of lagged kt issued there
                _q = []
                for kt in range(KT):
                    due = [l for l in _q if l[1] <= kt]
                    _q = [l for l in _q if l[1] > kt]
                    here = [l[0] for l in due]
                    if kt in DVE_KT or _CFG["lag_all"]:
                        _q.append((kt, kt + AV_LAG))
                    else:
                        here.append(kt)
                    lag_sched.append(here)
                    issue_seq.extend(here)
                tail_flush = [l[0] for l in _q]
                issue_seq.extend(tail_flush)
                first_av, last_av = issue_seq[0], issue_seq[-1]

                if _CFG["streams"] == 2 and phases != "front":
                    # Two groups interleaved slot-by-slot: each group's
                    # S->exp->AV chain gets a full slot-pair of slack while
                    # the other group keeps the engines fed. Same 8 PSUM
                    # banks (each group single-buffers S and AV).
                    groups = [(p, qc) for p in range(NPAIR)
                              for qc in range(QCN)]
                    pend = []

                    def s2_av(c, kt, start, stop):
                        p, qc = c["p"], c["qc"]
                        p_t = c.pop("pt_" + str(kt))
                        nc.tensor.matmul(
                            c["pAV"][0:65, 0:512],
                            v_sb[:, kt, p, 0, :], p_t[:, 0:512],
                            start=start, stop=stop)
                        nc.tensor.matmul(
                            c["pAV"][0:65, 512:1024],
                            v_sb[:, kt, p, 1, :], p_t[:, 512:1024],
                            start=start, stop=stop)

                    for blk in range(0, len(groups), 2):
                        ctxs = []
                        for (p, qc) in groups[blk:blk + 2]:
                            pAV = ps_av.tile(
                                [128, 1024], F32, tag="av", name="pAV")
                            ctxs.append({"p": p, "qc": qc, "pAV": pAV})
                        for kt in range(KT):
                            for c in ctxs:
                                p, qc = c["p"], c["qc"]
                                cs = slice(qc * 512, (qc + 1) * 512)
                                ks = slice(kt * 128, (kt + 1) * 128)
                                kT = kTf[:, p, :]
                                qT = qTf[:, p, :]
                                pS = ps_s.tile([128, 1024], F32, tag="s")
                                nc.tensor.matmul(
                                    pS[:, 0:512], kT[0:64, ks], qT[0:64, cs],
                                    start=True, stop=True,
                                    tile_position=(0, 0))
                                nc.tensor.matmul(
                                    pS[:, 512:1024], kT[64:128, ks],
                                    qT[64:128, cs],
                                    start=True, stop=True,
                                    tile_position=(64, 0))
                                c["pS"] = pS
                            if kt == 0 and pend:
                                emit_tail(*pend.pop(0))
                            if kt == 1 and pend:
                                emit_tail(*pend.pop(0))
                            for c in ctxs:
                                if kt > 0:
                                    s2_av(c, kt - 1, kt - 1 == 0, False)
                            for c in ctxs:
                                p_tile = p_pool.tile(
                                    [128, 1024], BF16, tag="P")
                                nc.scalar.activation(
                                    p_tile[:], c["pS"], AF.Exp, scale=_LN2)
                                c["pt_" + str(kt)] = p_tile[:]
                        for c in ctxs:
                            s2_av(c, KT - 1, False, True)
                        for c in ctxs:
                            rcp = emit_rcp(c["pAV"])
                            pend.append((c["p"], c["qc"], c["pAV"], rcp))
                    for t in pend:
                        emit_tail(*t)

                pending = None
                for p in range(NPAIR if (phases != "front"
                                         and _CFG["streams"] == 1) else 0):
                    qT = None if _CFG["s_merge"] else qTf[:, p, :]
                    kT = kTf[:, p, :]
                    for qc in range(QCN):
                        cs = slice(qc * 512, (qc + 1) * 512)
                        pAV = ps_av.tile([128, 1024], F32, tag="av")
                        p_aps = {}

                        def emit_av(kt):
                            st = kt == first_av
                            sp = kt == last_av
                            p0, p1 = p_aps.pop(kt)
                            nc.tensor.matmul(
                                pAV[0:65, 0:512],
                                v_sb[:, kt, p, 0, :], p0,
                                start=st, stop=sp)
                            if _CFG["av_single"]:
                                return
                            nc.tensor.matmul(
                                pAV[0:65, 512:1024],
                                v_sb[:, kt, p, 1, :], p1,
                                start=st, stop=sp)

                        for kt in range(KT):
                            ks = slice(kt * 128, (kt + 1) * 128)
                            pS = ps_s.tile([128, 1024], F32, tag="s")
                            if _CFG["no_s2"]:
                                nc.tensor.matmul(
                                    pS[:, 0:512], kT[0:64, ks], qT[0:64, cs],
                                    start=True, stop=True,
                                    tile_position=(0, 0))
                            elif _CFG["s_merge"]:
                                # one K=128 N=1024 matmul; the zero padding in
                                # qTp keeps the heads separate
                                nc.tensor.matmul(
                                    pS[:], kT[:, ks],
                                    qTp[:, p, qc, :, :].rearrange(
                                        "p a b -> p (a b)"),
                                    start=True, stop=True)
                            else:
                                nc.tensor.matmul(
                                    pS[:, 0:512], kT[0:64, ks], qT[0:64, cs],
                                    start=True, stop=True, tile_position=(0, 0))
                                nc.tensor.matmul(
                                    pS[:, 512:1024], kT[64:128, ks],
                                    qT[64:128, cs],
                                    start=True, stop=True,
                                    tile_position=(64, 0))
                            # S is in log2 domain (log2e folded into wq).
                            # ScalarE slots: exp(ln2*x) = 2^x via the free
                            # affine. DVE slots: Schraudolph 2^x -- int16
                            # round(128x + 16256 - C) bitcast to bf16 (max
                            # ~3.3% exp err on ~1/3 of key blocks; softmax
                            # denominators stay consistent via the ones
                            # column in v).
                            if kt in DVE_KT:
                                pi = pi_pool.tile([128, 1024], I16, tag="Pi")
                                nc.vector.tensor_scalar(
                                    pi[:], pS[:], 128.0, _SCHRAUDOLPH_BIAS,
                                    OP.mult, OP.add)
                                if _CFG["pi_copy"]:
                                    p_tile = p_pool.tile(
                                        [128, 1024], BF16, tag="P")
                                    nc.vector.tensor_copy(
                                        p_tile[:], pi[:].bitcast(BF16))
                                    pb = p_tile[:]
                                else:
                                    pb = pi[:].bitcast(BF16)
                                p_aps[kt] = (pb[:, 0:512], pb[:, 512:1024])
                            elif _CFG["exp_fd"] == 512:
                                # timing probe: half the ScalarE exp work
                                # (math wrong; head1 reuses head0's p)
                                p_tile = p_pool.tile([128, 1024], BF16, tag="P")
                                nc.scalar.activation(
                                    p_tile[:, 0:512], pS[:, 0:512],
                                    AF.Exp, scale=_LN2)
                                p_aps[kt] = (p_tile[:, 0:512],
                                             p_tile[:, 0:512])
                            else:
                                p_tile = p_pool.tile([128, 1024], BF16, tag="P")
                                nc.scalar.activation(
                                    p_tile[:], pS[:], AF.Exp, scale=_LN2)
                                p_aps[kt] = (p_tile[:, 0:512],
                                             p_tile[:, 512:1024])
                            # AV with ones-augmented v: rows 0:64 attn out,
                            # row 64 the softmax denominator (bank per head)
                            for lkt in lag_sched[kt]:
                                emit_av(lkt)
                            if kt == 4 and pending is not None:
                                emit_tail(*pending)
                                pending = None
                        for lkt in tail_flush:
                            emit_av(lkt)
                        rcp = emit_rcp(pAV)
                        pending = (p, qc, pAV, rcp)
                if pending is not None:
                    emit_tail(*pending)
                att_es.close()

                # ---- P4: out projection (accumulate over the 4 pairs)
                with tc.tile_pool(name="ps_o", bufs=3, space="PSUM") as ps_o, \
                     tc.tile_pool(name="wop", bufs=1) as wop, \
                     tc.tile_pool(name="osb", bufs=3) as osb:
                    wo_sb = wop.tile([128, NPAIR, D], BF16)
                    nc.sync.dma_start(wo_sb[:], wo_d[:])
                    for tt in range(NT):
                        if phases == "front":
                            # consume q/k/v so the front isn't dead code
                            qsrc = (qTp[:, tt % NPAIR, 0, 0, :]
                                    if _CFG["s_merge"] else
                                    qTf[:, tt % NPAIR, 0:512])
                            dm = osb.tile([128, 512], F32, tag="o")
                            nc.vector.tensor_tensor(
                                dm[:], qsrc,
                                kTf[:, tt % NPAIR, 0:512], OP.mult)
                            nc.vector.tensor_tensor(
                                dm[:, 0:65], v_sb[:, tt, 0, 0, :], dm[:, 0:65],
                                OP.mult)
                            nc.sync.dma_start(
                                out_d[tt * 128:(tt + 1) * 128, 0:512], dm[:])
                            continue
                        if phases != "all":
                            # consume attn_sb so attention isn't dead code
                            dm = osb.tile([128, 512], F32, tag="o")
                            nc.vector.tensor_copy(
                                dm[:], attn_sb[:, tt % NPAIR, 0:512])
                            nc.sync.dma_start(
                                out_d[tt * 128:(tt + 1) * 128, 0:512], dm[:])
                            continue
                        if _CFG["p4_pair"]:
                            # hf-inner: consecutive matmuls share lhsT (the
                            # attn pair tile) so the legalizer drops half the
                            # Ldweights
                            po0 = ps_o.tile([128, 512], F32, tag="o", name="po0")
                            po1 = ps_o.tile([128, 512], F32, tag="o", name="po1")
                            for p in range(NPAIR):
                                at = attn_sb[:, p, tt * 128:(tt + 1) * 128]
                                nc.tensor.matmul(
                                    po0[:], at, wo_sb[:, p, 0:512],
                                    start=(p == 0), stop=(p == NPAIR - 1))
                                nc.tensor.matmul(
                                    po1[:], at, wo_sb[:, p, 512:1024],
                                    start=(p == 0), stop=(p == NPAIR - 1))
                            for hf, po in ((0, po0), (1, po1)):
                                o_sb = osb.tile([128, 512], F32, tag="o")
                                if hf == 0:
                                    nc.vector.tensor_copy(o_sb[:], po[:])
                                else:
                                    nc.scalar.copy(o_sb[:], po[:])
                                nc.sync.dma_start(
                                    out_d[tt * 128:(tt + 1) * 128,
                                          hf * 512:(hf + 1) * 512], o_sb[:])
                            continue
                        if _CFG["p4_wide"]:
                            po = ps_o.tile([128, 1024], F32, tag="ow")
                            for p in range(NPAIR):
                                nc.tensor.matmul(
                                    po[:], attn_sb[:, p, tt * 128:(tt + 1) * 128],
                                    wo_sb[:, p, :],
                                    start=(p == 0), stop=(p == NPAIR - 1))
                            o_sb = osb.tile([128, 1024], F32, tag="ow")
                            if tt % 2 == 0:
                                nc.vector.tensor_copy(o_sb[:], po[:])
                            else:
                                nc.scalar.copy(o_sb[:], po[:])
                            nc.sync.dma_start(
                                out_d[tt * 128:(tt + 1) * 128, :], o_sb[:])
                            continue
                        for hf in range(2):
                            po = ps_o.tile([128, 512], F32, tag="o")
                            for p in range(NPAIR):
                                nc.tensor.matmul(
                                    po[:], attn_sb[:, p, tt * 128:(tt + 1) * 128],
                                    wo_sb[:, p, hf * 512:(hf + 1) * 512],
                                    start=(p == 0), stop=(p == NPAIR - 1))
                            o_sb = osb.tile([128, 512], F32, tag="o")
                            nc.vector.tensor_copy(o_sb[:], po[:])
                            nc.sync.dma_start(
                                out_d[tt * 128:(tt + 1) * 128, hf * 512:(hf + 1) * 512],
                                o_sb[:])
    return nc


def _get_program(loop_n=1, phases="all"):
    key = (loop_n, phases, _cfg_key())
    if key not in _PROG:
        _install_bir_patch()
        _PROG[key] = _build_program(loop_n, phases)
    return _PROG[key]


def _make_in_maps(x, gamma, w_qkv, w_out):
    x = np.asarray(x, dtype=np.float32)
    gamma = np.asarray(gamma, dtype=np.float32)
    w_qkv = np.asarray(w_qkv, dtype=np.float32)
    w_out = np.asarray(w_out, dtype=np.float32)

    scale = gamma * np.float32(np.sqrt(D))          # fold sqrt(D)*gamma
    in_maps = []
    for core in range(8):
        b = core // 2
        hg = core % 2
        cols = slice(hg * 512, (hg + 1) * 512)
        wq = (w_qkv[:, 0 * D:1 * D][:, cols] * scale[:, None]
              * np.float32(DH ** -0.5) * np.float32(np.log2(np.e)))
        wk = w_qkv[:, 1 * D:2 * D][:, cols] * scale[:, None]
        wv = w_qkv[:, 2 * D:3 * D][:, cols] * scale[:, None]
        wo = w_out[hg * 512:(hg + 1) * 512, :]
        import ml_dtypes
        bf16 = ml_dtypes.bfloat16
        in_maps.append({
            "x": np.ascontiguousarray(x[b]),
            "wq": np.ascontiguousarray(
                wq.reshape(FC, 128, 512).transpose(1, 0, 2).astype(bf16)),
            "wk": np.ascontiguousarray(
                wk.reshape(FC, 128, 512).transpose(1, 0, 2).astype(bf16)),
            "wv": np.ascontiguousarray(
                wv.reshape(FC, 128, 512).transpose(1, 0, 2).astype(bf16)),
            "wo": np.ascontiguousarray(
                wo.reshape(NPAIR, 128, D).transpose(1, 0, 2).astype(bf16)),
        })
    return in_maps


_RUNNER = None


def _build_runner(nc):
    """Persistent jitted callable over the 8-core mesh (avoids per-call
    re-tracing that run_bass_kernel_spmd incurs)."""
    import jax
    import concourse.mybir as mybir
    from jax.sharding import Mesh, PartitionSpec
    from jax.experimental.shard_map import shard_map
    from concourse.bass2jax import (
        _bass_exec_p, install_neuronx_cc_hook, partition_id_tensor)

    install_neuronx_cc_hook()
    partition_name = nc.partition_id_tensor.name if nc.partition_id_tensor else None
    in_names, out_names, out_avals, zero_shapes = [], [], [], []
    for alloc in nc.m.functions[0].allocations:
        if not isinstance(alloc, mybir.MemoryLocationSet):
            continue
        name = alloc.memorylocations[0].name
        if alloc.kind == "ExternalInput":
            if name != partition_name:
                in_names.append(name)
        elif alloc.kind == "ExternalOutput":
            out_names.append(name)
            shape = tuple(alloc.tensor_shape)
            dtype = mybir.dt.np(alloc.dtype)
            out_avals.append(jax.core.ShapedArray(shape, dtype))
            zero_shapes.append((shape, dtype))
    n_params = len(in_names)
    all_in_names = tuple(in_names + out_names)
    if partition_name is not None:
        all_in_names = all_in_names + (partition_name,)

    def _body(*args):
        operands = list(args)
        if partition_name is not None:
            operands.append(partition_id_tensor())
        return tuple(_bass_exec_p.bind(
            *operands,
            out_avals=tuple(out_avals),
            in_names=all_in_names,
            out_names=tuple(out_names),
            lowering_input_output_aliases=(),
            sim_require_finite=True,
            sim_require_nnan=True,
            nc=nc,
        ))

    devices = jax.devices()[:8]
    mesh = Mesh(np.asarray(devices), ("core",))
    nin = n_params + len(out_names)
    fn = jax.jit(shard_map(
        _body, mesh=mesh, in_specs=(PartitionSpec("core"),) * nin,
        out_specs=(PartitionSpec("core"),) * len(out_names), check_rep=False))

    def runner(in_maps):
        args = [np.concatenate([np.asarray(in_maps[c][nm]) for c in range(8)],
                               axis=0) for nm in in_names]
        for shape, dtype in zero_shapes:
            args.append(np.zeros((8 * shape[0], *shape[1:]), dtype))
        outs = fn(*args)
        o = np.asarray(outs[0]).reshape(8, T, D)
        return [o[c] for c in range(8)]

    return runner


def run(x, gamma, w_qkv, w_out, trace=False):
    """Run on the 8 NeuronCores; returns (output, results-or-None)."""
    global _RUNNER
    nc = _get_program()
    in_maps = _make_in_maps(x, gamma, w_qkv, w_out)
    res = None
    try:
        if _RUNNER is None:
            _RUNNER = _build_runner(nc)
        parts = _RUNNER(in_maps)
    except Exception:
        from concourse.bass_utils import run_bass_kernel_spmd
        res = run_bass_kernel_spmd(nc, in_maps, list(range(8)), trace=trace)
        parts = [res.results[i]["out"] for i in range(8)]
    out = np.stack([parts[2 * b] + parts[2 * b + 1] for b in range(B)], axis=0)
    return out, res


def kernel(x, gamma, w_qkv, w_out):
    out, _ = run(x, gamma, w_qkv, w_out, trace=False)
    return out

